# revision 1
# baseline (speedup 1.0000x reference)
"""Causal self-attention (B=2, S=2048, HID=1024, 16 heads x 64) on 8 trn2
NeuronCores.

Sharding: data-parallel over batch (cores 0-3 -> batch 0, cores 4-7 ->
batch 1), tensor-parallel over heads (4 heads per core via Wqk/Wv column
slices). Each core computes its 4 heads end-to-end; the [S, S] score
matrix stays core-local.

Per-core layout choices:
  - q, k are produced TRANSPOSED ([head_cols, S]) so score matmuls need
    no on-device transposes; scores are computed transposed ([sk, sq])
    so the P @ v matmul consumes exp(scores) directly from SBUF.
  - v carries an appended ones-column per head; the attention output
    matmul then yields softmax row-sums in an extra partition row for
    free (no max-subtraction is needed: scores are O(5) so exp is safe
    in fp32, and masked entries are zeroed multiplicatively post-exp).
  - All matmuls run in float32r (fp32 data, PE fast path).
  - Heads are processed in pairs: the two K=64 score matmuls sit in PE
    row-groups 0-63 / 64-127 and run concurrently in the array.
  - The second half (S columns 1024:2048) of the q/k/v projections is
    emitted interleaved with the attention over stripes a=0,1 (which
    only need the first half), so the tensor engine never idles while
    the scalar engine works through the exp() stream -- idle windows
    re-throttle the PE clock to 1.2 GHz (HAM).
"""
import sys

for _p in ("/opt/trn_rl_repo",):
    if _p not in sys.path:
        sys.path.insert(0, _p)

import numpy as np

B, S, HID = 2, 2048, 1024
NH, HD = 16, 64
NHL = 4            # heads per core
WC = NHL * HD      # 256 local q/k weight cols
VC = NHL * (HD + 1)  # 260 local v cols incl. ones col
NT = S // 128      # 16 key chunks
NA = S // 512      # 4 query stripes
NK = HID // 128    # 8 contraction chunks

_NC = None


def _build():
    from concourse import bacc, mybir
    from concourse.tile import TileContext
    from concourse.masks import make_identity

    FP = mybir.dt.float32
    FPR = mybir.dt.float32r
    Exp = mybir.ActivationFunctionType.Exp

    nc = bacc.Bacc("TRN2", target_bir_lowering=False, debug=False, num_devices=8)

    xT = nc.dram_tensor("xT", [HID, S], FPR, kind="ExternalInput")
    wq = nc.dram_tensor("wq", [HID, WC], FPR, kind="ExternalInput")
    wk = nc.dram_tensor("wk", [HID, WC], FPR, kind="ExternalInput")
    wv = nc.dram_tensor("wv", [HID + 1, VC], FPR, kind="ExternalInput")
    bq = nc.dram_tensor("bq", [WC, 1], FP, kind="ExternalInput")
    bk = nc.dram_tensor("bk", [WC, 1], FP, kind="ExternalInput")
    ones = nc.dram_tensor("ones", [1, 128], FPR, kind="ExternalInput")
    out = nc.dram_tensor("out", [S, WC], FP, kind="ExternalOutput")

    with TileContext(nc) as tc:
        with (
            tc.tile_pool(name="inp", bufs=1) as inp,
            tc.tile_pool(name="ptp", bufs=3) as ptp,
            tc.tile_pool(name="osb", bufs=16) as osb,
            tc.tile_pool(name="rcp", bufs=4) as rcp,
            tc.tile_pool(name="onat", bufs=8) as onp,
            tc.tile_pool(name="G", bufs=3, space="PSUM") as gp,
            tc.tile_pool(name="oT", bufs=2, space="PSUM") as otp,
        ):
            # ---- persistent inputs in SBUF ----
            # tiny tensors first: they gate the first projection epilogues
            bq_sb = [inp.tile([128, 1], FP, name=f"bq{t}") for t in range(2)]
            bk_sb = [inp.tile([128, 1], FP, name=f"bk{t}") for t in range(2)]
            for t in range(2):
                nc.sync.dma_start(bq_sb[t][:, :], bq[t * 128:(t + 1) * 128, :])
                nc.sync.dma_start(bk_sb[t][:, :], bk[t * 128:(t + 1) * 128, :])
            ones1 = inp.tile([1, 128], FPR, name="ones1")
            nc.sync.dma_start(ones1[:, :], ones[:, :])
            ident = inp.tile([128, 128], FP, name="ident")
            make_identity(nc, ident[:, :])
            # interleave wq/wk/x-quarter-0 per K-chunk: the first projection
            # units' K-chains consume chunks in this exact order, so the PE
            # starts after ~1 chunk instead of after the full 5MB
            wq_k = [inp.tile([128, WC], FPR, name=f"wq{k}") for k in range(NK)]
            wk_k = [inp.tile([128, WC], FPR, name=f"wk{k}") for k in range(NK)]
            xk = [[None] * 4 for _ in range(NK)]
            for k in range(NK):
                nc.sync.dma_start(wq_k[k][:, :], wq[k * 128:(k + 1) * 128, :])
                nc.sync.dma_start(wk_k[k][:, :], wk[k * 128:(k + 1) * 128, :])
                t = inp.tile([128, 512], FPR, name=f"x{k}_0")
                nc.sync.dma_start(t[:, :], xT[k * 128:(k + 1) * 128, 0:512])
                xk[k][0] = t
            # v weights next: they gate the out matmuls of stripe 0
            wv_k = [inp.tile([128, VC], FPR, name=f"wv{k}") for k in range(NK)]
            for k in range(NK):
                nc.sync.dma_start(wv_k[k][:, :], wv[k * 128:(k + 1) * 128, :])
            wv_last = inp.tile([1, VC], FPR, name="wvl")
            nc.sync.dma_start(wv_last[:, :], wv[HID:HID + 1, :])
            for qtr in range(1, 4):
                for k in range(NK):
                    t = inp.tile([128, 512], FPR, name=f"x{k}_{qtr}")
                    nc.sync.dma_start(
                        t[:, :], xT[k * 128:(k + 1) * 128, qtr * 512:(qtr + 1) * 512]
                    )
                    xk[k][qtr] = t
            # split by S-quarter so interleaved later-quarter projection
            # writes can't false-depend against earlier attention reads
            qT_sb = [[inp.tile([128, 512], FPR, name=f"qT{t}_{n}")
                      for n in range(4)] for t in range(2)]
            kT_sb = [[inp.tile([128, 512], FPR, name=f"kT{t}_{n}")
                      for n in range(4)] for t in range(2)]
            v_sb = [inp.tile([128, VC], FPR, name=f"v{c}") for c in range(NT)]

            # ---- projection emitters ----
            def proj_qk_unit(wt, bt, dst, t, qtr):
                g = gp.tile([128, 1024], mybir.dt.float32, tag="G", name="g")
                for k in range(NK):
                    nc.tensor.matmul(
                        g[:, :512],
                        lhsT=wt[k][:, t * 128:(t + 1) * 128],
                        rhs=xk[k][qtr][:, :],
                        start=(k == 0), stop=(k == NK - 1),
                    )
                nc.vector.tensor_scalar_add(
                    dst[t][qtr][:, :], g[:, :512], bt[t][:, :]
                )

            def proj_v_unit(c):
                qtr, cc = divmod(c, 4)
                g = gp.tile([128, 1024], mybir.dt.float32, tag="G", name="g")
                for k in range(NK):
                    nc.tensor.matmul(
                        g[:, :VC],
                        lhsT=xk[k][qtr][:, cc * 128:(cc + 1) * 128],
                        rhs=wv_k[k][:, :],
                        start=(k == 0), stop=False,
                    )
                nc.tensor.matmul(  # bias row + ones column (K=1)
                    g[:, :VC], lhsT=ones1[:, :], rhs=wv_last[:, :],
                    start=False, stop=True,
                )
                nc.vector.tensor_copy(v_sb[c][:, :], g[:, :VC])

            # ---- attention emitters ----
            # unit = ONE key chunk b for a head PAIR: g = [h0-slice | h1-slice],
            # one exp covers both heads; fine granularity keeps 3 chunks in
            # flight within the 6 PSUM banks of the G pool
            def att_unit(a, ht, b, nchunks, oTs):
                g = gp.tile([128, 1024], mybir.dt.float32, tag="G", name="g")
                kn, ko = divmod(b * 128, 512)
                # diagonal chunks: columns < off are fully masked -- skip
                # them in the score matmul and the exp, memset them instead
                off = max(0, (b - 4 * a) * 128)
                for hh in range(2):
                    hb = hh * 64
                    nc.tensor.matmul(
                        g[:, hh * 512 + off:(hh + 1) * 512],
                        lhsT=kT_sb[ht][kn][hb:hb + 64, ko:ko + 128],
                        rhs=qT_sb[ht][a][hb:hb + 64, off:],
                        start=True, stop=True,
                    )
                pt = ptp.tile([128, 1024], FPR, tag="pt", name="pt")
                if off:
                    gv = g[:, :].rearrange("p (h w) -> p h w", h=2)[:, :, off:]
                    pv = pt[:, :].rearrange("p (h w) -> p h w", h=2)[:, :, off:]
                    nc.scalar.activation(pv, gv, Exp, scale=HD ** -0.5)
                else:
                    nc.scalar.activation(pt[:, :], g[:, :], Exp, scale=HD ** -0.5)
                if b >= 4 * a:
                    # zeroes the skipped stale prefix (condition always false
                    # there) and the triangular boundary block in one select
                    for hh in range(2):
                        nc.gpsimd.affine_select(
                            out=pt[:, hh * 512:hh * 512 + off + 128],
                            in_=pt[:, hh * 512:hh * 512 + off + 128],
                            compare_op=mybir.AluOpType.is_ge,
                            fill=0.0, base=a * 512 - b * 128,
                            pattern=[[1, off + 128]], channel_multiplier=-1,
                        )
                for hh in range(2):
                    h = 2 * ht + hh
                    nc.tensor.matmul(
                        oTs[hh][:, :],
                        lhsT=v_sb[b][:, h * 65:(h + 1) * 65],
                        rhs=pt[:, hh * 512:(hh + 1) * 512],
                        start=(b == 0), stop=(b == nchunks - 1),
                    )

            def finish_head(a, ht, hh, oT_sb, onat):
                h = 2 * ht + hh
                for c in range(4):
                    tr = gp.tile([128, HD + 1], mybir.dt.float32,
                                 tag="G", name="tr")
                    nc.tensor.transpose(
                        tr[:, :HD + 1], oT_sb[:, c * 128:(c + 1) * 128],
                        ident[:HD + 1, :HD + 1],
                    )
                    recip = rcp.tile([128, 1], FP, tag="recip", name="recip")
                    nc.vector.reciprocal(recip[:, :], tr[:, HD:HD + 1])
                    nc.vector.tensor_scalar_mul(
                        onat[c][:, h * 64:(h + 1) * 64], tr[:, :HD], recip[:, :]
                    )

            # ---- phase 1: the minimum needed by stripe a=0 head pair 0 ----
            proj_qk_unit(wq_k, bq_sb, qT_sb, 0, 0)
            proj_qk_unit(wk_k, bk_sb, kT_sb, 0, 0)
            proj_v_unit(0)
            proj_v_unit(1)

            # remaining projection units are doled out between attention
            # units, scheduled (just) before their first consumer, keeping
            # the PE busy while ACT works through the exp stream
            def q_(t, qtr):
                return lambda: proj_qk_unit(wq_k, bq_sb, qT_sb, t, qtr)

            def k_(t, qtr):
                return lambda: proj_qk_unit(wk_k, bk_sb, kT_sb, t, qtr)

            def v_(c):
                return lambda: proj_v_unit(c)

            done_heads = {}
            onat_by_a = {}

            def fin_(a, ht, hh):
                def run():
                    if a not in onat_by_a:
                        onat_by_a[a] = [
                            onp.tile([128, WC], FP, tag="onat", name="onat")
                            for _ in range(4)]
                    finish_head(a, ht, hh, done_heads[(a, ht, hh)], onat_by_a[a])
                return run

            def dma_(a):
                def run():
                    for c in range(4):
                        r = (a * 4 + c) * 128
                        nc.sync.dma_start(out[r:r + 128, :],
                                          onat_by_a[a][c][:, :])
                return run

            filler = {
                0: [v_(2)], 1: [v_(3)], 2: [q_(1, 0)], 3: [k_(1, 0)],
                4: [q_(0, 1)], 5: [k_(0, 1)], 6: [v_(4)], 7: [v_(5)],
                8: [v_(6)], 9: [v_(7)], 10: [q_(1, 1)], 12: [k_(1, 1)],
                14: [q_(0, 2)], 17: [k_(0, 2)], 20: [q_(1, 2)], 23: [v_(8)],
                26: [v_(9)], 28: [v_(10)], 30: [v_(11)], 32: [k_(1, 2)],
                34: [q_(0, 3)], 38: [k_(0, 3)], 42: [v_(12)], 46: [v_(13)],
                50: [v_(14)], 54: [v_(15)], 58: [q_(1, 3)], 62: [k_(1, 3)],
            }

            # ---- phases 2+3: attention (head tails deferred) ----
            uidx = 0
            for a in range(NA):
                nchunks = 4 * a + 4
                for ht in range(2):
                    oTs = [otp.tile([HD + 1, 512], mybir.dt.float32,
                                    tag="oT", name="oT") for _ in range(2)]
                    for b in range(nchunks):
                        att_unit(a, ht, b, nchunks, oTs)
                        for f in filler.get(uidx, ()):
                            f()
                        uidx += 1
                    # drain oT psum quickly so the next head pair can start
                    for hh in range(2):
                        oT_sb = osb.tile([HD + 1, 512], FP, tag="oTsb",
                                         name="oTsb")
                        nc.vector.tensor_copy(oT_sb[:, :], oTs[hh][:, :])
                        done_heads[(a, ht, hh)] = oT_sb

            # ---- tail: finish + store all stripes ----
            for a in range(NA):
                for ht in range(2):
                    for hh in range(2):
                        fin_(a, ht, hh)()
                dma_(a)()

    nc.compile()
    return nc


def _get_nc():
    global _NC
    if _NC is None:
        _NC = _build()
    return _NC


def make_in_maps(hidden_states, Wqk, bqk, Wv, bv):
    x = np.ascontiguousarray(np.asarray(hidden_states, dtype=np.float32))
    Wqk = np.asarray(Wqk, dtype=np.float32)
    bqk = np.asarray(bqk, dtype=np.float32)
    Wv = np.asarray(Wv, dtype=np.float32)
    bv = np.asarray(bv, dtype=np.float32)

    xTs = [np.ascontiguousarray(x[b].T) for b in range(B)]
    in_maps = []
    for c in range(8):
        b, ho = c // 4, (c % 4) * NHL
        cols = slice(ho * HD, (ho + NHL) * HD)
        wv_aug = np.zeros((HID + 1, VC), np.float32)
        for h in range(NHL):
            wv_aug[:HID, h * 65:h * 65 + HD] = Wv[:, (ho + h) * HD:(ho + h + 1) * HD]
            wv_aug[HID, h * 65:h * 65 + HD] = bv[(ho + h) * HD:(ho + h + 1) * HD]
            wv_aug[HID, h * 65 + HD] = 1.0
        in_maps.append({
            "xT": xTs[b],
            "wq": np.ascontiguousarray(Wqk[:, cols]),
            "wk": np.ascontiguousarray(Wqk[:, HID:][:, cols]),
            "wv": wv_aug,
            "bq": np.ascontiguousarray(bqk[:HID][cols].reshape(WC, 1)),
            "bk": np.ascontiguousarray(bqk[HID:][cols].reshape(WC, 1)),
            "ones": np.ones((1, 128), np.float32),
        })
    return in_maps


def kernel(hidden_states, Wqk, bqk, Wv, bv):
    import time

    from concourse.bass_utils import run_bass_kernel_spmd

    in_maps = make_in_maps(hidden_states, Wqk, bqk, Wv, bv)
    res = None
    for attempt in range(3):
        try:
            res = run_bass_kernel_spmd(_get_nc(), in_maps, list(range(8)))
            break
        except Exception:
            # transient NRT_EXEC_UNIT_UNRECOVERABLE errors have been observed
            # on this fabric; back off and retry
            if attempt == 2:
                raise
            time.sleep(2.0)
    outp = np.empty((B, S, NH * HD), np.float32)
    for c in range(8):
        b, ho = c // 4, (c % 4) * NHL
        outp[b, :, ho * HD:(ho + NHL) * HD] = res.results[c]["out"]
    return outp



# revision 4
# speedup vs baseline: 1.0877x; 1.0877x over previous
"""Causal self-attention (B=2, S=2048, HID=1024, 16 heads x 64) on 8 trn2
NeuronCores.

Sharding: data-parallel over batch (cores 0-3 -> batch 0, cores 4-7 ->
batch 1), tensor-parallel over heads (4 heads per core via Wqk/Wv column
slices). Each core computes its 4 heads end-to-end; the [S, S] score
matrix stays core-local.

Per-core layout choices:
  - All matmul operands are bf16 (inputs are cast host-side): the PE
    streams 1 col/cycle at 2.4 GHz and FWL halves LDWEIGHTS time; fp32
    paths measured ~2x slower on HW. PSUM accumulation stays fp32.
  - q, k are produced TRANSPOSED ([head_cols, S]) so score matmuls need
    no on-device transposes; scores are computed transposed ([sk, sq])
    so the P @ v matmul consumes exp(scores) directly from SBUF.
  - v carries an appended ones-column per head; the attention output
    matmul then yields softmax row-sums in an extra partition row for
    free (no max-subtraction is needed: scores are O(5) so exp is safe
    in fp32, and masked entries are zeroed multiplicatively post-exp
    with a DVE multiply against a precomputed [128,128] triangle mask).
  - Heads are processed in pairs: the two K=64 score matmuls sit in PE
    row-groups 0-63 / 64-127 and run concurrently in the array.
  - Attention matmuls skip fully-masked column prefixes on diagonal
    chunks (both the exp and the P @ v matmul are restricted).
  - A warmup burst of identity matmuls runs during the input DMA so the
    HAM clock gate reaches 8/8 before the real stream starts; head
    finalization (transpose + normalize + store) is interleaved with
    the attention stream so the PE never idles into a re-throttle.
"""
import sys

for _p in ("/opt/trn_rl_repo",):
    if _p not in sys.path:
        sys.path.insert(0, _p)

import numpy as np

B, S, HID = 2, 2048, 1024
NH, HD = 16, 64
NHL = 4            # heads per core
WC = NHL * HD      # 256 local q/k weight cols
VC = NHL * (HD + 1)  # 260 local v cols incl. ones col
NT = S // 128      # 16 key chunks
NA = S // 512      # 4 query stripes
NK = HID // 128    # 8 contraction chunks

_NC = None


def _build():
    from concourse import bacc, mybir
    from concourse.tile import TileContext
    from concourse.masks import make_identity, make_upper_triangular

    FP = mybir.dt.float32
    BF = mybir.dt.bfloat16
    Exp = mybir.ActivationFunctionType.Exp

    nc = bacc.Bacc("TRN2", target_bir_lowering=False, debug=False, num_devices=8)

    xT = nc.dram_tensor("xT", [HID, S], BF, kind="ExternalInput")
    wq = nc.dram_tensor("wq", [HID, WC], BF, kind="ExternalInput")
    wk = nc.dram_tensor("wk", [HID, WC], BF, kind="ExternalInput")
    wv = nc.dram_tensor("wv", [HID + 1, VC], BF, kind="ExternalInput")
    bq = nc.dram_tensor("bq", [WC, 1], FP, kind="ExternalInput")
    bk = nc.dram_tensor("bk", [WC, 1], FP, kind="ExternalInput")
    ones = nc.dram_tensor("ones", [1, 128], BF, kind="ExternalInput")
    out = nc.dram_tensor("out", [S, WC], FP, kind="ExternalOutput")

    with TileContext(nc) as tc:
        with (
            tc.tile_pool(name="inp", bufs=1) as inp,
            tc.tile_pool(name="ptp", bufs=3) as ptp,
            tc.tile_pool(name="osb", bufs=8) as osb,
            tc.tile_pool(name="rcp", bufs=4) as rcp,
            tc.tile_pool(name="onat", bufs=8) as onp,
            tc.tile_pool(name="G", bufs=3, space="PSUM") as gp,
            tc.tile_pool(name="oT", bufs=2, space="PSUM") as otp,
        ):
            # ---- persistent inputs in SBUF ----
            # tiny tensors first: they gate the first projection epilogues
            bq_sb = [inp.tile([128, 1], FP, name=f"bq{t}") for t in range(2)]
            bk_sb = [inp.tile([128, 1], FP, name=f"bk{t}") for t in range(2)]
            for t in range(2):
                nc.sync.dma_start(bq_sb[t][:, :], bq[t * 128:(t + 1) * 128, :])
                nc.sync.dma_start(bk_sb[t][:, :], bk[t * 128:(t + 1) * 128, :])
            ones1 = inp.tile([1, 128], BF, name="ones1")
            nc.sync.dma_start(ones1[:, :], ones[:, :])
            ident = inp.tile([128, 128], BF, name="ident")
            make_identity(nc, ident[:, :])
            # triangle mask for diagonal chunks: tri[p, j] = 1.0 if j >= p
            tri = inp.tile([128, 128], BF, name="tri")
            make_upper_triangular(nc, tri[:, :], val=1.0, diag=True)

            # PE warmup: identity matmuls keep the PE busy through the HAM
            # SHORT window while the input DMA streams, so the projection
            # stream starts at 2.4 GHz instead of 1.2
            warm = gp.tile([128, 1024], mybir.dt.float32, tag="G", name="warm")
            for _ in range(40):
                nc.tensor.matmul(warm[:, :128], lhsT=ident[:, :],
                                 rhs=ident[:, :], start=True, stop=True)

            # interleave wq/wk/x-quarter-0 per K-chunk: the first projection
            # units' K-chains consume chunks in this exact order, so the PE
            # starts after ~1 chunk instead of after the full payload
            wq_k = [inp.tile([128, WC], BF, name=f"wq{k}") for k in range(NK)]
            wk_k = [inp.tile([128, WC], BF, name=f"wk{k}") for k in range(NK)]
            xk = [[None] * 4 for _ in range(NK)]
            for k in range(NK):
                nc.sync.dma_start(wq_k[k][:, :], wq[k * 128:(k + 1) * 128, :])
                nc.sync.dma_start(wk_k[k][:, :], wk[k * 128:(k + 1) * 128, :])
                t = inp.tile([128, 512], BF, name=f"x{k}_0")
                nc.sync.dma_start(t[:, :], xT[k * 128:(k + 1) * 128, 0:512])
                xk[k][0] = t
            # v weights next: they gate the out matmuls of stripe 0
            wv_k = [inp.tile([128, VC], BF, name=f"wv{k}") for k in range(NK)]
            for k in range(NK):
                nc.sync.dma_start(wv_k[k][:, :], wv[k * 128:(k + 1) * 128, :])
            wv_last = inp.tile([1, VC], BF, name="wvl")
            nc.sync.dma_start(wv_last[:, :], wv[HID:HID + 1, :])
            for qtr in range(1, 4):
                for k in range(NK):
                    t = inp.tile([128, 512], BF, name=f"x{k}_{qtr}")
                    nc.sync.dma_start(
                        t[:, :], xT[k * 128:(k + 1) * 128, qtr * 512:(qtr + 1) * 512]
                    )
                    xk[k][qtr] = t
            # split by S-quarter so interleaved later-quarter projection
            # writes can't false-depend against earlier attention reads
            qT_sb = [[inp.tile([128, 512], BF, name=f"qT{t}_{n}")
                      for n in range(4)] for t in range(2)]
            kT_sb = [[inp.tile([128, 512], BF, name=f"kT{t}_{n}")
                      for n in range(4)] for t in range(2)]
            v_sb = [inp.tile([128, VC], BF, name=f"v{c}") for c in range(NT)]

            # ---- projection emitters ----
            def proj_qk_unit(wt, bt, dst, t, qtr):
                g = gp.tile([128, 1024], mybir.dt.float32, tag="G", name="g")
                for k in range(NK):
                    nc.tensor.matmul(
                        g[:, :512],
                        lhsT=wt[k][:, t * 128:(t + 1) * 128],
                        rhs=xk[k][qtr][:, :],
                        start=(k == 0), stop=(k == NK - 1),
                    )
                nc.vector.tensor_scalar_add(
                    dst[t][qtr][:, :], g[:, :512], bt[t][:, :]
                )

            def proj_v_unit(c):
                qtr, cc = divmod(c, 4)
                g = gp.tile([128, 1024], mybir.dt.float32, tag="G", name="g")
                for k in range(NK):
                    nc.tensor.matmul(
                        g[:, :VC],
                        lhsT=xk[k][qtr][:, cc * 128:(cc + 1) * 128],
                        rhs=wv_k[k][:, :],
                        start=(k == 0), stop=False,
                    )
                nc.tensor.matmul(  # bias row + ones column (K=1)
                    g[:, :VC], lhsT=ones1[:, :], rhs=wv_last[:, :],
                    start=False, stop=True,
                )
                nc.vector.tensor_copy(v_sb[c][:, :], g[:, :VC])

            # ---- attention emitters ----
            # unit = ONE key chunk b for a head PAIR: g = [h0-slice | h1-slice],
            # one exp covers both heads; fine granularity keeps 3 chunks in
            # flight within the 6 PSUM banks of the G pool
            def att_unit(a, ht, b, nchunks, oTs):
                g = gp.tile([128, 1024], mybir.dt.float32, tag="G", name="g")
                kn, ko = divmod(b * 128, 512)
                # diagonal chunks: columns < off are fully masked -- skip
                # them in the score matmul, the exp, and the P @ v matmul
                off = max(0, (b - 4 * a) * 128)
                for hh in range(2):
                    hb = hh * 64
                    nc.tensor.matmul(
                        g[:, hh * 512 + off:(hh + 1) * 512],
                        lhsT=kT_sb[ht][kn][hb:hb + 64, ko:ko + 128],
                        rhs=qT_sb[ht][a][hb:hb + 64, off:],
                        start=True, stop=True,
                    )
                pt = ptp.tile([128, 1024], BF, tag="pt", name="pt")
                if off:
                    gv = g[:, :].rearrange("p (h w) -> p h w", h=2)[:, :, off:]
                    pv = pt[:, :].rearrange("p (h w) -> p h w", h=2)[:, :, off:]
                    nc.scalar.activation(pv, gv, Exp, scale=HD ** -0.5)
                else:
                    nc.scalar.activation(pt[:, :], g[:, :], Exp, scale=HD ** -0.5)
                if b >= 4 * a:
                    # triangular boundary block: multiplicative mask on DVE
                    for hh in range(2):
                        c0 = hh * 512 + off
                        nc.vector.tensor_mul(
                            pt[:, c0:c0 + 128], pt[:, c0:c0 + 128], tri[:, :]
                        )
                for hh in range(2):
                    h = 2 * ht + hh
                    nc.tensor.matmul(
                        oTs[hh][:, off:],
                        lhsT=v_sb[b][:, h * 65:(h + 1) * 65],
                        rhs=pt[:, hh * 512 + off:(hh + 1) * 512],
                        start=(b == 0), stop=(b == nchunks - 1),
                    )

            def finish_head(a, ht, hh, oT_sb, onat):
                h = 2 * ht + hh
                for c in range(4):
                    tr = gp.tile([128, HD + 1], BF, tag="G", name="tr")
                    nc.tensor.transpose(
                        tr[:, :HD + 1], oT_sb[:, c * 128:(c + 1) * 128],
                        ident[:HD + 1, :HD + 1],
                    )
                    recip = rcp.tile([128, 1], FP, tag="recip", name="recip")
                    nc.vector.reciprocal(recip[:, :], tr[:, HD:HD + 1])
                    nc.vector.tensor_scalar_mul(
                        onat[c][:, h * 64:(h + 1) * 64], tr[:, :HD], recip[:, :]
                    )

            # ---- phase 1: the minimum needed by stripe a=0 head pair 0 ----
            proj_qk_unit(wq_k, bq_sb, qT_sb, 0, 0)
            proj_qk_unit(wk_k, bk_sb, kT_sb, 0, 0)
            proj_v_unit(0)
            proj_v_unit(1)

            # remaining projection units are doled out between attention
            # units, scheduled (just) before their first consumer, keeping
            # the PE busy while ACT works through the exp stream
            def q_(t, qtr):
                return lambda: proj_qk_unit(wq_k, bq_sb, qT_sb, t, qtr)

            def k_(t, qtr):
                return lambda: proj_qk_unit(wk_k, bk_sb, kT_sb, t, qtr)

            def v_(c):
                return lambda: proj_v_unit(c)

            onat_by_a = {}

            def ensure_onat(a):
                if a not in onat_by_a:
                    onat_by_a[a] = [
                        onp.tile([128, WC], FP, tag="onat", name="onat")
                        for _ in range(4)]
                return onat_by_a[a]

            filler = {
                0: [v_(2)], 1: [v_(3)], 2: [q_(1, 0)], 3: [k_(1, 0)],
                4: [q_(0, 1)], 5: [k_(0, 1)], 6: [v_(4)], 7: [v_(5)],
                8: [v_(6)], 9: [v_(7)], 10: [q_(1, 1)], 12: [k_(1, 1)],
                14: [q_(0, 2)], 17: [k_(0, 2)], 20: [q_(1, 2)], 23: [v_(8)],
                26: [v_(9)], 28: [v_(10)], 30: [v_(11)], 32: [k_(1, 2)],
                34: [q_(0, 3)], 38: [k_(0, 3)], 42: [v_(12)], 46: [v_(13)],
                50: [v_(14)], 54: [v_(15)], 58: [q_(1, 3)], 62: [k_(1, 3)],
            }

            # ---- phases 2+3: attention with interleaved head finish ----
            uidx = 0
            for a in range(NA):
                nchunks = 4 * a + 4
                for ht in range(2):
                    oTs = [otp.tile([HD + 1, 512], mybir.dt.float32,
                                    tag="oT", name="oT") for _ in range(2)]
                    for b in range(nchunks):
                        att_unit(a, ht, b, nchunks, oTs)
                        for f in filler.get(uidx, ()):
                            f()
                        uidx += 1
                    onat = ensure_onat(a)
                    # drain oT psum quickly so the next head pair can start,
                    # then finish (transpose + normalize) in-stream
                    for hh in range(2):
                        oT_sb = osb.tile([HD + 1, 512], BF, tag="oTsb",
                                         name="oTsb")
                        nc.vector.tensor_copy(oT_sb[:, :], oTs[hh][:, :])
                        finish_head(a, ht, hh, oT_sb, onat)
                # store stripe a as soon as both head pairs finished
                for c in range(4):
                    r = (a * 4 + c) * 128
                    nc.sync.dma_start(out[r:r + 128, :], onat_by_a[a][c][:, :])

    nc.compile()
    return nc


def _get_nc():
    global _NC
    if _NC is None:
        _NC = _build()
    return _NC


def make_in_maps(hidden_states, Wqk, bqk, Wv, bv):
    from ml_dtypes import bfloat16

    x = np.asarray(hidden_states, dtype=np.float32)
    Wqk = np.asarray(Wqk, dtype=np.float32)
    bqk = np.asarray(bqk, dtype=np.float32)
    Wv = np.asarray(Wv, dtype=np.float32)
    bv = np.asarray(bv, dtype=np.float32)

    xTs = [np.ascontiguousarray(x[b].T.astype(bfloat16)) for b in range(B)]
    in_maps = []
    for c in range(8):
        b, ho = c // 4, (c % 4) * NHL
        cols = slice(ho * HD, (ho + NHL) * HD)
        wv_aug = np.zeros((HID + 1, VC), np.float32)
        for h in range(NHL):
            wv_aug[:HID, h * 65:h * 65 + HD] = Wv[:, (ho + h) * HD:(ho + h + 1) * HD]
            wv_aug[HID, h * 65:h * 65 + HD] = bv[(ho + h) * HD:(ho + h + 1) * HD]
            wv_aug[HID, h * 65 + HD] = 1.0
        in_maps.append({
            "xT": xTs[b],
            "wq": np.ascontiguousarray(Wqk[:, cols].astype(bfloat16)),
            "wk": np.ascontiguousarray(Wqk[:, HID:][:, cols].astype(bfloat16)),
            "wv": wv_aug.astype(bfloat16),
            "bq": np.ascontiguousarray(bqk[:HID][cols].reshape(WC, 1)),
            "bk": np.ascontiguousarray(bqk[HID:][cols].reshape(WC, 1)),
            "ones": np.ones((1, 128), bfloat16),
        })
    return in_maps


def kernel(hidden_states, Wqk, bqk, Wv, bv):
    import time

    from concourse.bass_utils import run_bass_kernel_spmd

    in_maps = make_in_maps(hidden_states, Wqk, bqk, Wv, bv)
    res = None
    for attempt in range(3):
        try:
            res = run_bass_kernel_spmd(_get_nc(), in_maps, list(range(8)))
            break
        except Exception:
            # transient NRT_EXEC_UNIT_UNRECOVERABLE errors have been observed
            # on this fabric; back off and retry
            if attempt == 2:
                raise
            time.sleep(2.0)
    outp = np.empty((B, S, NH * HD), np.float32)
    for c in range(8):
        b, ho = c // 4, (c % 4) * NHL
        outp[b, :, ho * HD:(ho + NHL) * HD] = res.results[c]["out"]
    return outp


# revision 7
# speedup vs baseline: 1.2509x; 1.1501x over previous
"""Causal self-attention (B=2, S=2048, HID=1024, 16 heads x 64) on 8 trn2
NeuronCores.

Sharding: data-parallel over batch (cores 0-3 -> batch 0, cores 4-7 ->
batch 1), tensor-parallel over heads (4 heads per core via Wqk/Wv column
slices). Each core computes its 4 heads end-to-end; the [S, S] score
matrix stays core-local.

Per-core layout choices:
  - All matmul operands are bf16 (inputs are cast host-side): the PE
    streams 1 col/cycle at 2.4 GHz and FWL halves LDWEIGHTS time; fp32
    paths measured ~2x slower on HW. PSUM accumulation stays fp32.
  - q, k are produced TRANSPOSED ([head_cols, S]) so score matmuls need
    no on-device transposes; scores are computed transposed ([sk, sq])
    so the P @ v matmul consumes exp(scores) directly from SBUF.
  - v carries an appended ones-column per head; the attention output
    matmul then yields softmax row-sums in an extra partition row for
    free (no max-subtraction is needed: scores are O(5) so exp is safe
    in fp32, and masked entries are zeroed multiplicatively post-exp
    with a DVE multiply against a [128,128] triangle mask input).
  - Heads are processed in pairs: the two K=64 score matmuls sit in PE
    row-groups 0-63 / 64-127 and run concurrently in the array.
  - The P @ v matmuls run 2 chunks BEHIND the score/exp stream, so the
    in-order PE queue never stalls on the ACT engine's exp latency.
  - Inputs arrive in ~13 large DMAs (the Sync engine costs ~600ns per
    dma_start, so many small DMAs serialize the front); identity and
    triangle-mask constants are DMA inputs, so GPSIMD (with its ~6us
    first-call IRAM load) is never touched.
  - A warmup burst of identity matmuls runs during the input DMA so the
    HAM clock gate reaches 8/8 before the real stream starts; head
    finalization (transpose + normalize + store) is deferred into the
    following head-pair's stream, keeping the PE dense end-to-end.
"""
import sys

for _p in ("/opt/trn_rl_repo",):
    if _p not in sys.path:
        sys.path.insert(0, _p)

import numpy as np

B, S, HID = 2, 2048, 1024
NH, HD = 16, 64
NHL = 4            # heads per core
WC = NHL * HD      # 256 local q/k weight cols
VC = NHL * (HD + 1)  # 260 local v cols incl. ones col
NT = S // 128      # 16 key chunks
NA = S // 512      # 4 query stripes
NK = HID // 128    # 8 contraction chunks
LAG = 2            # P @ v trails the score/exp stream by this many chunks

_NC = None


def _build():
    from concourse import bacc, mybir
    from concourse.tile import TileContext

    FP = mybir.dt.float32
    BF = mybir.dt.bfloat16
    Exp = mybir.ActivationFunctionType.Exp

    nc = bacc.Bacc("TRN2", target_bir_lowering=False, debug=False, num_devices=8)

    xT = nc.dram_tensor("xT", [HID, S], BF, kind="ExternalInput")
    wq = nc.dram_tensor("wq", [HID, WC], BF, kind="ExternalInput")
    wk = nc.dram_tensor("wk", [HID, WC], BF, kind="ExternalInput")
    wv = nc.dram_tensor("wv", [HID + 1, VC], BF, kind="ExternalInput")
    bqk = nc.dram_tensor("bqk", [2 * WC, 1], FP, kind="ExternalInput")
    ident_d = nc.dram_tensor("ident", [128, 128], BF, kind="ExternalInput")
    tri_d = nc.dram_tensor("tri", [128, 128], BF, kind="ExternalInput")
    out = nc.dram_tensor("out", [S, WC], FP, kind="ExternalOutput")

    with TileContext(nc) as tc:
        with (
            tc.tile_pool(name="inp", bufs=1) as inp,
            tc.tile_pool(name="ptp", bufs=4) as ptp,
            tc.tile_pool(name="osb", bufs=8) as osb,
            tc.tile_pool(name="rcp", bufs=4) as rcp,
            tc.tile_pool(name="onat", bufs=8) as onp,
            tc.tile_pool(name="G", bufs=3, space="PSUM") as gp,
            tc.tile_pool(name="oT", bufs=2, space="PSUM") as otp,
        ):
            # ---- persistent inputs in SBUF, few large DMAs ----
            ident = inp.tile([128, 128], BF, name="ident")
            nc.sync.dma_start(ident[:, :], ident_d[:, :])
            tri = inp.tile([128, 128], BF, name="tri")
            nc.sync.dma_start(tri[:, :], tri_d[:, :])
            bqk_sb = inp.tile([128, 4], FP, name="bqk")
            nc.sync.dma_start(
                bqk_sb[:, :].rearrange("p (t j) -> p t j", j=1),
                bqk[:, :].rearrange("(t p) j -> p t j", p=128),
            )
            # wq/wk whole-tensor: [1024, 256] -> [128, 8*256] (k-major cols)
            wq_sb = inp.tile([128, NK * WC], BF, name="wq")
            wk_sb = inp.tile([128, NK * WC], BF, name="wk")
            for dst, src in ((wq_sb, wq), (wk_sb, wk)):
                nc.sync.dma_start(
                    dst[:, :].rearrange("p (k j) -> p k j", k=NK),
                    src[:, :].rearrange("(k p) j -> p k j", k=NK),
                )
            # x quarter 0 first (gates the first projections)
            xq = [None] * 4
            xq[0] = inp.tile([128, NK * 512], BF, name="xq0")
            nc.sync.dma_start(
                xq[0][:, :].rearrange("p (k j) -> p k j", k=NK),
                xT[:, 0:512].rearrange("(k p) j -> p k j", k=NK),
            )
            # v weights next: they gate the out matmuls of stripe 0
            wv_sb = inp.tile([128, NK * VC], BF, name="wv")
            nc.sync.dma_start(
                wv_sb[:, :].rearrange("p (k j) -> p k j", k=NK),
                wv[0:HID, :].rearrange("(k p) j -> p k j", k=NK),
            )
            wv_last = inp.tile([1, VC], BF, name="wvl")
            nc.sync.dma_start(wv_last[:, :], wv[HID:HID + 1, :])
            for qtr in range(1, 4):
                xq[qtr] = inp.tile([128, NK * 512], BF, name=f"xq{qtr}")
                nc.sync.dma_start(
                    xq[qtr][:, :].rearrange("p (k j) -> p k j", k=NK),
                    xT[:, qtr * 512:(qtr + 1) * 512].rearrange(
                        "(k p) j -> p k j", k=NK),
                )

            def xk(k, qtr):
                return xq[qtr][:, k * 512:(k + 1) * 512]

            # PE warmup: identity matmuls keep the PE busy through the HAM
            # SHORT window while the input DMA streams, so the projection
            # stream starts at 2.4 GHz instead of 1.2
            warm = gp.tile([128, 1024], mybir.dt.float32, tag="G", name="warm")
            for _ in range(36):
                nc.tensor.matmul(warm[:, :128], lhsT=ident[:, :],
                                 rhs=ident[:, :], start=True, stop=True)

            # split by S-quarter so interleaved later-quarter projection
            # writes can't false-depend against earlier attention reads
            qT_sb = [[inp.tile([128, 512], BF, name=f"qT{t}_{n}")
                      for n in range(4)] for t in range(2)]
            kT_sb = [[inp.tile([128, 512], BF, name=f"kT{t}_{n}")
                      for n in range(4)] for t in range(2)]
            v_sb = [inp.tile([128, VC], BF, name=f"v{c}") for c in range(NT)]

            # ---- projection emitters ----
            def proj_qk_unit(wt, bcol, dst, t, qtr):
                g = gp.tile([128, 1024], mybir.dt.float32, tag="G", name="g")
                for k in range(NK):
                    nc.tensor.matmul(
                        g[:, :512],
                        lhsT=wt[:, k * WC + t * 128:k * WC + (t + 1) * 128],
                        rhs=xk(k, qtr),
                        start=(k == 0), stop=(k == NK - 1),
                    )
                nc.vector.tensor_scalar_add(
                    dst[t][qtr][:, :], g[:, :512], bqk_sb[:, bcol + t:bcol + t + 1]
                )

            def proj_v_unit(c):
                qtr, cc = divmod(c, 4)
                g = gp.tile([128, 1024], mybir.dt.float32, tag="G", name="g")
                for k in range(NK):
                    nc.tensor.matmul(
                        g[:, :VC],
                        lhsT=xk(k, qtr)[:, cc * 128:(cc + 1) * 128],
                        rhs=wv_sb[:, k * VC:(k + 1) * VC],
                        start=(k == 0), stop=False,
                    )
                nc.tensor.matmul(  # bias row + ones column (K=1)
                    # tri row 0 is all-ones: broadcasts wv_last to all rows
                    g[:, :VC], lhsT=tri[0:1, 0:128], rhs=wv_last[:, :],
                    start=False, stop=True,
                )
                nc.vector.tensor_copy(v_sb[c][:, :], g[:, :VC])

            # ---- attention emitters ----
            # score/exp for ONE key chunk b of a head PAIR:
            # g = [h0-slice | h1-slice], one exp covers both heads
            def score_exp_unit(a, ht, b):
                g = gp.tile([128, 1024], mybir.dt.float32, tag="G", name="g")
                kn, ko = divmod(b * 128, 512)
                # diagonal chunks: columns < off are fully masked -- skip
                # them in the score matmul, the exp, and the P @ v matmul
                off = max(0, (b - 4 * a) * 128)
                for hh in range(2):
                    hb = hh * 64
                    nc.tensor.matmul(
                        g[:, hh * 512 + off:(hh + 1) * 512],
                        lhsT=kT_sb[ht][kn][hb:hb + 64, ko:ko + 128],
                        rhs=qT_sb[ht][a][hb:hb + 64, off:],
                        start=True, stop=True,
                    )
                pt = ptp.tile([128, 1024], BF, tag="pt", name="pt")
                if off:
                    gv = g[:, :].rearrange("p (h w) -> p h w", h=2)[:, :, off:]
                    pv = pt[:, :].rearrange("p (h w) -> p h w", h=2)[:, :, off:]
                    nc.scalar.activation(pv, gv, Exp, scale=HD ** -0.5)
                else:
                    nc.scalar.activation(pt[:, :], g[:, :], Exp, scale=HD ** -0.5)
                if b >= 4 * a:
                    # triangular boundary block: multiplicative mask on DVE
                    for hh in range(2):
                        c0 = hh * 512 + off
                        nc.vector.tensor_mul(
                            pt[:, c0:c0 + 128], pt[:, c0:c0 + 128], tri[:, :]
                        )
                return pt

            def av_unit(a, ht, b, nchunks, oTs, pt):
                off = max(0, (b - 4 * a) * 128)
                for hh in range(2):
                    h = 2 * ht + hh
                    nc.tensor.matmul(
                        oTs[hh][:, off:],
                        lhsT=v_sb[b][:, h * 65:(h + 1) * 65],
                        rhs=pt[:, hh * 512 + off:(hh + 1) * 512],
                        start=(b == 0), stop=(b == nchunks - 1),
                    )

            def finish_head(a, ht, hh, oT_sb, onat):
                h = 2 * ht + hh
                for c in range(4):
                    tr = gp.tile([128, HD + 1], BF, tag="G", name="tr")
                    nc.tensor.transpose(
                        tr[:, :HD + 1], oT_sb[:, c * 128:(c + 1) * 128],
                        ident[:HD + 1, :HD + 1],
                    )
                    recip = rcp.tile([128, 1], FP, tag="recip", name="recip")
                    nc.vector.reciprocal(recip[:, :], tr[:, HD:HD + 1])
                    nc.vector.tensor_scalar_mul(
                        onat[:, c * WC + h * 64:c * WC + (h + 1) * 64],
                        tr[:, :HD], recip[:, :]
                    )

            # ---- phase 1: the minimum needed by stripe a=0 head pair 0 ----
            proj_qk_unit(wq_sb, 0, qT_sb, 0, 0)
            proj_qk_unit(wk_sb, 2, kT_sb, 0, 0)
            proj_v_unit(0)
            proj_v_unit(1)

            # remaining projection units are doled out between attention
            # units, scheduled (just) before their first consumer, keeping
            # the PE busy while ACT works through the exp stream
            def q_(t, qtr):
                return lambda: proj_qk_unit(wq_sb, 0, qT_sb, t, qtr)

            def k_(t, qtr):
                return lambda: proj_qk_unit(wk_sb, 2, kT_sb, t, qtr)

            def v_(c):
                return lambda: proj_v_unit(c)

            # placement: just-before-first-consumer deadlines, spread so
            # every region keeps the PE slightly ahead of the exp stream
            # (pair tails especially: the flush P @ v matmuls wait there)
            filler = {
                0: [v_(2)], 1: [v_(3)], 2: [q_(1, 0)], 3: [k_(1, 0)],
                4: [q_(0, 1)], 5: [k_(0, 1)], 6: [v_(4)], 7: [v_(5)],
                9: [v_(6)], 11: [v_(7)], 13: [q_(1, 1)], 15: [k_(1, 1)],
                17: [q_(0, 2)], 20: [k_(0, 2)], 22: [v_(8)], 23: [v_(9)],
                26: [v_(10)], 29: [v_(11)], 32: [q_(1, 2)], 35: [k_(1, 2)],
                39: [q_(0, 3)], 43: [k_(0, 3)], 50: [v_(12)], 53: [q_(1, 3)],
                54: [v_(13)], 57: [k_(1, 3)], 58: [v_(14)], 61: [v_(15)],
            }

            onat_by_a = {}
            deferred = []          # finish/store closures fed into the stream

            # ---- phases 2+3: attention, software-pipelined ----
            uidx = 0
            for a in range(NA):
                nchunks = 4 * a + 4
                if a not in onat_by_a:
                    onat_by_a[a] = onp.tile([128, 4 * WC], FP, tag="onat",
                                            name="onat")
                for ht in range(2):
                    oTs = [otp.tile([HD + 1, 512], mybir.dt.float32,
                                    tag="oT", name="oT") for _ in range(2)]
                    pend = []
                    for b in range(nchunks):
                        pend.append((b, score_exp_unit(a, ht, b)))
                        if len(pend) > LAG:
                            bb, pt = pend.pop(0)
                            av_unit(a, ht, bb, nchunks, oTs, pt)
                        for f in filler.get(uidx, ()):
                            f()
                        if deferred:
                            deferred.pop(0)()
                        uidx += 1
                    for bb, pt in pend:
                        av_unit(a, ht, bb, nchunks, oTs, pt)
                    # drain oT psum immediately so the next pair's first
                    # P @ v (LAG units away) finds its slots free
                    for hh in range(2):
                        oT_sb = osb.tile([HD + 1, 512], BF, tag="oTsb",
                                         name="oTsb")
                        nc.vector.tensor_copy(oT_sb[:, :], oTs[hh][:, :])
                        deferred.append(
                            (lambda a_=a, ht_=ht, hh_=hh, t_=oT_sb:
                             finish_head(a_, ht_, hh_, t_, onat_by_a[a_])))
                # store stripe a once its finishes have run
                def store(a_=a):
                    nc.sync.dma_start(
                        out[a_ * 512:(a_ + 1) * 512, :].rearrange(
                            "(c p) j -> p c j", p=128),
                        onat_by_a[a_][:, :].rearrange("p (c j) -> p c j", c=4),
                    )
                deferred.append(store)
            for f in deferred:
                f()

    nc.compile()
    return nc


def _get_nc():
    global _NC
    if _NC is None:
        _NC = _build()
    return _NC


def make_in_maps(hidden_states, Wqk, bqk, Wv, bv):
    from ml_dtypes import bfloat16

    x = np.asarray(hidden_states, dtype=np.float32)
    Wqk = np.asarray(Wqk, dtype=np.float32)
    bqk = np.asarray(bqk, dtype=np.float32)
    Wv = np.asarray(Wv, dtype=np.float32)
    bv = np.asarray(bv, dtype=np.float32)

    ident = np.eye(128, dtype=bfloat16)
    tri = np.triu(np.ones((128, 128), np.float32)).astype(bfloat16)
    xTs = [np.ascontiguousarray(x[b].T.astype(bfloat16)) for b in range(B)]
    in_maps = []
    for c in range(8):
        b, ho = c // 4, (c % 4) * NHL
        cols = slice(ho * HD, (ho + NHL) * HD)
        wv_aug = np.zeros((HID + 1, VC), np.float32)
        for h in range(NHL):
            wv_aug[:HID, h * 65:h * 65 + HD] = Wv[:, (ho + h) * HD:(ho + h + 1) * HD]
            wv_aug[HID, h * 65:h * 65 + HD] = bv[(ho + h) * HD:(ho + h + 1) * HD]
            wv_aug[HID, h * 65 + HD] = 1.0
        bqk_c = np.concatenate([bqk[:HID][cols], bqk[HID:][cols]])
        in_maps.append({
            "xT": xTs[b],
            "wq": np.ascontiguousarray(Wqk[:, cols].astype(bfloat16)),
            "wk": np.ascontiguousarray(Wqk[:, HID:][:, cols].astype(bfloat16)),
            "wv": wv_aug.astype(bfloat16),
            "bqk": np.ascontiguousarray(bqk_c.reshape(2 * WC, 1)),
            "ident": ident,
            "tri": tri,
        })
    return in_maps


def kernel(hidden_states, Wqk, bqk, Wv, bv):
    import time

    from concourse.bass_utils import run_bass_kernel_spmd

    in_maps = make_in_maps(hidden_states, Wqk, bqk, Wv, bv)
    res = None
    for attempt in range(3):
        try:
            res = run_bass_kernel_spmd(_get_nc(), in_maps, list(range(8)))
            break
        except Exception:
            # transient NRT_EXEC_UNIT_UNRECOVERABLE errors have been observed
            # on this fabric; back off and retry
            if attempt == 2:
                raise
            time.sleep(2.0)
    outp = np.empty((B, S, NH * HD), np.float32)
    for c in range(8):
        b, ho = c // 4, (c % 4) * NHL
        outp[b, :, ho * HD:(ho + NHL) * HD] = res.results[c]["out"]
    return outp


# revision 10
# speedup vs baseline: 1.3414x; 1.0723x over previous
"""Causal self-attention (B=2, S=2048, HID=1024, 16 heads x 64) on 8 trn2
NeuronCores.

Sharding: data-parallel over batch (cores 0-3 -> batch 0, cores 4-7 ->
batch 1), tensor-parallel over heads (4 heads per core via Wqk/Wv column
slices). Each core computes its 4 heads end-to-end; the [S, S] score
matrix stays core-local.

Per-core layout choices:
  - All matmul operands are bf16 (inputs are cast host-side): the PE
    streams 1 col/cycle at 2.4 GHz and FWL halves LDWEIGHTS time; fp32
    paths measured ~2x slower on HW. PSUM accumulation stays fp32.
  - q, k are produced TRANSPOSED ([head_cols, S]) so score matmuls need
    no on-device transposes; scores are computed transposed ([sk, sq])
    so the P @ v matmul consumes exp(scores) directly from SBUF.
  - v carries an appended ones-column per head; the attention output
    matmul then yields softmax row-sums in an extra partition row for
    free (no max-subtraction is needed: scores are O(5) so exp is safe
    in fp32, and masked entries are zeroed multiplicatively post-exp
    with a DVE multiply against a [128,128] triangle mask input).
  - Heads are processed in pairs: the two K=64 score matmuls sit in PE
    row-groups 0-63 / 64-127 and run concurrently in the array.
  - The P @ v matmuls run 2 chunks BEHIND the score/exp stream, so the
    in-order PE queue never stalls on the ACT engine's exp latency.
  - Inputs arrive in ~13 large DMAs (the Sync engine costs ~600ns per
    dma_start, so many small DMAs serialize the front); identity and
    triangle-mask constants are DMA inputs, so GPSIMD (with its ~6us
    first-call IRAM load) is never touched.
  - A warmup burst of identity matmuls runs during the input DMA so the
    HAM clock gate reaches 8/8 before the real stream starts; head
    finalization (transpose + normalize + store) is deferred into the
    following head-pair's stream, keeping the PE dense end-to-end.
"""
import sys

for _p in ("/opt/trn_rl_repo",):
    if _p not in sys.path:
        sys.path.insert(0, _p)

import numpy as np

B, S, HID = 2, 2048, 1024
NH, HD = 16, 64
NHL = 4            # heads per core
WC = NHL * HD      # 256 local q/k weight cols
VC = NHL * (HD + 1)  # 260 local v cols incl. ones col
NT = S // 128      # 16 key chunks
NA = S // 512      # 4 query stripes
NK = HID // 128    # 8 contraction chunks
LAG = 2            # P @ v trails the score/exp stream by this many chunks

_NC = None


def _build():
    from concourse import bacc, mybir
    from concourse.tile import TileContext

    FP = mybir.dt.float32
    BF = mybir.dt.bfloat16
    Exp = mybir.ActivationFunctionType.Exp

    nc = bacc.Bacc("TRN2", target_bir_lowering=False, debug=False, num_devices=8)

    xT = nc.dram_tensor("xT", [HID, S], BF, kind="ExternalInput")
    wq = nc.dram_tensor("wq", [HID, WC], BF, kind="ExternalInput")
    wk = nc.dram_tensor("wk", [HID, WC], BF, kind="ExternalInput")
    wv = nc.dram_tensor("wv", [HID + 1, VC], BF, kind="ExternalInput")
    bqk = nc.dram_tensor("bqk", [2 * WC, 1], FP, kind="ExternalInput")
    ident_d = nc.dram_tensor("ident", [128, 128], BF, kind="ExternalInput")
    tri_d = nc.dram_tensor("tri", [128, 128], BF, kind="ExternalInput")
    out = nc.dram_tensor("out", [S, WC], FP, kind="ExternalOutput")

    with TileContext(nc) as tc:
        with (
            tc.tile_pool(name="inp", bufs=1) as inp,
            tc.tile_pool(name="ptp", bufs=4) as ptp,
            tc.tile_pool(name="osb", bufs=8) as osb,
            tc.tile_pool(name="rcp", bufs=4) as rcp,
            tc.tile_pool(name="onat", bufs=4) as onp,
            tc.tile_pool(name="G", bufs=3, space="PSUM") as gp,
            tc.tile_pool(name="oT", bufs=2, space="PSUM") as otp,
        ):
            # PE warmup on a zeroed scratch tile (no DMA dependency, so it
            # starts right after the engine preambles): keeps the PE busy
            # through the HAM SHORT window while the input DMA streams, so
            # the projection stream starts at 2.4 GHz instead of 1.2
            scratch = inp.tile([128, 512], BF, name="scratch")
            nc.vector.memset(scratch[:, :], 0.0)
            warm = gp.tile([128, 1024], mybir.dt.float32, tag="G", name="warm")
            for _ in range(30):
                nc.tensor.matmul(warm[:, :512], lhsT=scratch[:, :128],
                                 rhs=scratch[:, :], start=True, stop=True)

            # ---- persistent inputs in SBUF, few large DMAs split across
            # the two HWDGE issue queues (Sync + Scalar) ----
            ident = inp.tile([128, 128], BF, name="ident")
            nc.sync.dma_start(ident[:, :], ident_d[:, :])
            tri = inp.tile([128, 128], BF, name="tri")
            nc.scalar.dma_start(tri[:, :], tri_d[:, :])
            # wq/wk whole-tensor: [1024, 256] -> [128, 8*256] (k-major cols)
            wq_sb = inp.tile([128, NK * WC], BF, name="wq")
            nc.sync.dma_start(
                wq_sb[:, :].rearrange("p (k j) -> p k j", k=NK),
                wq[:, :].rearrange("(k p) j -> p k j", k=NK),
            )
            # v weights on the scalar queue: they gate stripe 0's out matmuls
            wv_sb = inp.tile([128, NK * VC], BF, name="wv")
            nc.scalar.dma_start(
                wv_sb[:, :].rearrange("p (k j) -> p k j", k=NK),
                wv[0:HID, :].rearrange("(k p) j -> p k j", k=NK),
            )
            # x quarter 0 in two halves (earlier first consumption)
            xq = [[None, None] for _ in range(4)]
            for h in range(2):
                t = inp.tile([128, 4 * 512], BF, name=f"xq0_{h}")
                nc.sync.dma_start(
                    t[:, :].rearrange("p (k j) -> p k j", k=4),
                    xT[h * 512:(h + 1) * 512, 0:512].rearrange(
                        "(k p) j -> p k j", k=4),
                )
                xq[0][h] = t
            wk_sb = inp.tile([128, NK * WC], BF, name="wk")
            nc.sync.dma_start(
                wk_sb[:, :].rearrange("p (k j) -> p k j", k=NK),
                wk[:, :].rearrange("(k p) j -> p k j", k=NK),
            )
            wv_last = inp.tile([1, VC], BF, name="wvl")
            nc.sync.dma_start(wv_last[:, :], wv[HID:HID + 1, :])
            bqk_sb = inp.tile([128, 4], FP, name="bqk")
            nc.sync.dma_start(
                bqk_sb[:, :].rearrange("p (t j) -> p t j", j=1),
                bqk[:, :].rearrange("(t p) j -> p t j", p=128),
            )
            for qtr in range(1, 4):
                t = inp.tile([128, NK * 512], BF, name=f"xq{qtr}")
                nc.scalar.dma_start(
                    t[:, :].rearrange("p (k j) -> p k j", k=NK),
                    xT[:, qtr * 512:(qtr + 1) * 512].rearrange(
                        "(k p) j -> p k j", k=NK),
                )
                xq[qtr][0] = t

            def xk(k, qtr):
                if qtr == 0:
                    t = xq[0][k // 4]
                    return t[:, (k % 4) * 512:(k % 4 + 1) * 512]
                return xq[qtr][0][:, k * 512:(k + 1) * 512]

            # split by S-quarter so interleaved later-quarter projection
            # writes can't false-depend against earlier attention reads
            qT_sb = [[inp.tile([128, 512], BF, name=f"qT{t}_{n}")
                      for n in range(4)] for t in range(2)]
            kT_sb = [[inp.tile([128, 512], BF, name=f"kT{t}_{n}")
                      for n in range(4)] for t in range(2)]
            v_sb = [inp.tile([128, VC], BF, name=f"v{c}") for c in range(NT)]

            # ---- projection emitters ----
            def proj_qk_unit(wt, bcol, dst, t, qtr):
                g = gp.tile([128, 1024], mybir.dt.float32, tag="G", name="g")
                for k in range(NK):
                    nc.tensor.matmul(
                        g[:, :512],
                        lhsT=wt[:, k * WC + t * 128:k * WC + (t + 1) * 128],
                        rhs=xk(k, qtr),
                        start=(k == 0), stop=(k == NK - 1),
                    )
                nc.vector.tensor_scalar_add(
                    dst[t][qtr][:, :], g[:, :512], bqk_sb[:, bcol + t:bcol + t + 1]
                )

            def proj_v_unit(c):
                qtr, cc = divmod(c, 4)
                g = gp.tile([128, 1024], mybir.dt.float32, tag="G", name="g")
                for k in range(NK):
                    nc.tensor.matmul(
                        g[:, :VC],
                        lhsT=xk(k, qtr)[:, cc * 128:(cc + 1) * 128],
                        rhs=wv_sb[:, k * VC:(k + 1) * VC],
                        start=(k == 0), stop=False,
                    )
                nc.tensor.matmul(  # bias row + ones column (K=1)
                    # tri row 0 is all-ones: broadcasts wv_last to all rows
                    g[:, :VC], lhsT=tri[0:1, 0:128], rhs=wv_last[:, :],
                    start=False, stop=True,
                )
                nc.vector.tensor_copy(v_sb[c][:, :], g[:, :VC])

            # ---- attention emitters ----
            # score/exp for ONE key chunk b of a head PAIR:
            # g = [h0-slice | h1-slice], one exp covers both heads
            def score_exp_unit(a, ht, b):
                g = gp.tile([128, 1024], mybir.dt.float32, tag="G", name="g")
                kn, ko = divmod(b * 128, 512)
                # diagonal chunks: columns < off are fully masked -- skip
                # them in the score matmul, the exp, and the P @ v matmul
                off = max(0, (b - 4 * a) * 128)
                for hh in range(2):
                    hb = hh * 64
                    nc.tensor.matmul(
                        g[:, hh * 512 + off:(hh + 1) * 512],
                        lhsT=kT_sb[ht][kn][hb:hb + 64, ko:ko + 128],
                        rhs=qT_sb[ht][a][hb:hb + 64, off:],
                        start=True, stop=True,
                    )
                pt = ptp.tile([128, 1024], BF, tag="pt", name="pt")
                if off:
                    gv = g[:, :].rearrange("p (h w) -> p h w", h=2)[:, :, off:]
                    pv = pt[:, :].rearrange("p (h w) -> p h w", h=2)[:, :, off:]
                    nc.scalar.activation(pv, gv, Exp, scale=HD ** -0.5)
                else:
                    nc.scalar.activation(pt[:, :], g[:, :], Exp, scale=HD ** -0.5)
                if b >= 4 * a:
                    # triangular boundary block: multiplicative mask on DVE
                    for hh in range(2):
                        c0 = hh * 512 + off
                        nc.vector.tensor_mul(
                            pt[:, c0:c0 + 128], pt[:, c0:c0 + 128], tri[:, :]
                        )
                return pt

            def av_unit(a, ht, b, nchunks, oTs, pt):
                off = max(0, (b - 4 * a) * 128)
                for hh in range(2):
                    h = 2 * ht + hh
                    nc.tensor.matmul(
                        oTs[hh][:, off:],
                        lhsT=v_sb[b][:, h * 65:(h + 1) * 65],
                        rhs=pt[:, hh * 512 + off:(hh + 1) * 512],
                        start=(b == 0), stop=(b == nchunks - 1),
                    )

            def finish_piece(a, ht, hh, c, oT_sb):
                # one 128-query block of one head: transpose + normalize.
                # single-transpose granularity spreads the (HAM-invisible)
                # PE transposes thinly through the stream
                h = 2 * ht + hh
                onat = onat_by_a[a]
                tr = gp.tile([128, HD + 1], BF, tag="G", name="tr")
                nc.tensor.transpose(
                    tr[:, :HD + 1], oT_sb[:, c * 128:(c + 1) * 128],
                    ident[:HD + 1, :HD + 1],
                )
                recip = rcp.tile([128, 1], FP, tag="recip", name="recip")
                nc.vector.reciprocal(recip[:, :], tr[:, HD:HD + 1])
                nc.vector.tensor_scalar_mul(
                    onat[:, c * WC + h * 64:c * WC + (h + 1) * 64],
                    tr[:, :HD], recip[:, :]
                )

            # ---- phase 1: the minimum needed by stripe a=0 head pair 0 ----
            proj_qk_unit(wq_sb, 0, qT_sb, 0, 0)
            proj_qk_unit(wk_sb, 2, kT_sb, 0, 0)
            proj_v_unit(0)
            proj_v_unit(1)

            # remaining projection units are doled out between attention
            # units, scheduled (just) before their first consumer, keeping
            # the PE busy while ACT works through the exp stream
            def q_(t, qtr):
                return lambda: proj_qk_unit(wq_sb, 0, qT_sb, t, qtr)

            def k_(t, qtr):
                return lambda: proj_qk_unit(wk_sb, 2, kT_sb, t, qtr)

            def v_(c):
                return lambda: proj_v_unit(c)

            # placement: just-before-first-consumer deadlines, spread so
            # every region keeps the PE slightly ahead of the exp stream
            # (pair tails especially: the flush P @ v matmuls wait there)
            filler = {
                0: [v_(2)], 1: [v_(3)], 2: [q_(1, 0)], 3: [k_(1, 0)],
                4: [q_(0, 1)], 5: [k_(0, 1)], 6: [v_(4)], 7: [v_(5)],
                9: [v_(6)], 11: [v_(7)], 13: [q_(1, 1)], 15: [k_(1, 1)],
                17: [q_(0, 2)], 20: [k_(0, 2)], 22: [v_(8)], 23: [v_(9)],
                26: [v_(10)], 29: [v_(11)], 32: [q_(1, 2)], 35: [k_(1, 2)],
                39: [q_(0, 3)], 43: [k_(0, 3)], 50: [v_(12)], 53: [q_(1, 3)],
                54: [v_(13)], 57: [k_(1, 3)], 58: [v_(14)], 61: [v_(15)],
            }

            onat_by_a = {}
            deferred = []          # finish/store closures fed into the stream

            # ---- phases 2+3: attention, software-pipelined ----
            uidx = 0
            for a in range(NA):
                nchunks = 4 * a + 4
                if a not in onat_by_a:
                    onat_by_a[a] = onp.tile([128, 4 * WC], FP, tag="onat",
                                            name="onat")
                for ht in range(2):
                    oTs = [otp.tile([HD + 1, 512], mybir.dt.float32,
                                    tag="oT", name="oT") for _ in range(2)]
                    pend = []
                    for b in range(nchunks):
                        pend.append((b, score_exp_unit(a, ht, b)))
                        if len(pend) > LAG:
                            bb, pt = pend.pop(0)
                            av_unit(a, ht, bb, nchunks, oTs, pt)
                        for f in filler.get(uidx, ()):
                            f()
                        if deferred:
                            deferred.pop(0)()
                        uidx += 1
                    for bb, pt in pend:
                        av_unit(a, ht, bb, nchunks, oTs, pt)
                    # drain oT psum immediately so the next pair's first
                    # P @ v (LAG units away) finds its slots free
                    for hh in range(2):
                        oT_sb = osb.tile([HD + 1, 512], BF, tag="oTsb",
                                         name="oTsb")
                        nc.vector.tensor_copy(oT_sb[:, :], oTs[hh][:, :])
                        for c in range(4):
                            deferred.append(
                                (lambda a_=a, ht_=ht, hh_=hh, c_=c, t_=oT_sb:
                                 finish_piece(a_, ht_, hh_, c_, t_)))

                    # store this head pair once its finish pieces have run
                    def store(a_=a, ht_=ht):
                        nc.sync.dma_start(
                            out[a_ * 512:(a_ + 1) * 512,
                                ht_ * 128:(ht_ + 1) * 128].rearrange(
                                "(c p) j -> p c j", p=128),
                            onat_by_a[a_][:, :].rearrange(
                                "p (c j) -> p c j", c=4)[
                                :, :, ht_ * 128:(ht_ + 1) * 128],
                        )
                    deferred.append(store)
            for f in deferred:
                f()

    nc.compile()
    return nc


def _get_nc():
    global _NC
    if _NC is None:
        _NC = _build()
    return _NC


def make_in_maps(hidden_states, Wqk, bqk, Wv, bv):
    from ml_dtypes import bfloat16

    x = np.asarray(hidden_states, dtype=np.float32)
    Wqk = np.asarray(Wqk, dtype=np.float32)
    bqk = np.asarray(bqk, dtype=np.float32)
    Wv = np.asarray(Wv, dtype=np.float32)
    bv = np.asarray(bv, dtype=np.float32)

    ident = np.eye(128, dtype=bfloat16)
    tri = np.triu(np.ones((128, 128), np.float32)).astype(bfloat16)
    xTs = [np.ascontiguousarray(x[b].T.astype(bfloat16)) for b in range(B)]
    in_maps = []
    for c in range(8):
        b, ho = c // 4, (c % 4) * NHL
        cols = slice(ho * HD, (ho + NHL) * HD)
        wv_aug = np.zeros((HID + 1, VC), np.float32)
        for h in range(NHL):
            wv_aug[:HID, h * 65:h * 65 + HD] = Wv[:, (ho + h) * HD:(ho + h + 1) * HD]
            wv_aug[HID, h * 65:h * 65 + HD] = bv[(ho + h) * HD:(ho + h + 1) * HD]
            wv_aug[HID, h * 65 + HD] = 1.0
        bqk_c = np.concatenate([bqk[:HID][cols], bqk[HID:][cols]])
        in_maps.append({
            "xT": xTs[b],
            "wq": np.ascontiguousarray(Wqk[:, cols].astype(bfloat16)),
            "wk": np.ascontiguousarray(Wqk[:, HID:][:, cols].astype(bfloat16)),
            "wv": wv_aug.astype(bfloat16),
            "bqk": np.ascontiguousarray(bqk_c.reshape(2 * WC, 1)),
            "ident": ident,
            "tri": tri,
        })
    return in_maps


def kernel(hidden_states, Wqk, bqk, Wv, bv):
    import time

    from concourse.bass_utils import run_bass_kernel_spmd

    in_maps = make_in_maps(hidden_states, Wqk, bqk, Wv, bv)
    res = None
    for attempt in range(3):
        try:
            res = run_bass_kernel_spmd(_get_nc(), in_maps, list(range(8)))
            break
        except Exception:
            # transient NRT_EXEC_UNIT_UNRECOVERABLE errors have been observed
            # on this fabric; back off and retry
            if attempt == 2:
                raise
            time.sleep(2.0)
    outp = np.empty((B, S, NH * HD), np.float32)
    for c in range(8):
        b, ho = c // 4, (c % 4) * NHL
        outp[b, :, ho * HD:(ho + NHL) * HD] = res.results[c]["out"]
    return outp


# revision 13
# speedup vs baseline: 1.3704x; 1.0216x over previous
"""Causal self-attention (B=2, S=2048, HID=1024, 16 heads x 64) on 8 trn2
NeuronCores.

Sharding: data-parallel over batch (cores 0-3 -> batch 0, cores 4-7 ->
batch 1), tensor-parallel over heads (4 heads per core via Wqk/Wv column
slices). Each core computes its 4 heads end-to-end; the [S, S] score
matrix stays core-local.

Per-core layout choices:
  - All matmul operands are bf16 (inputs are cast host-side): the PE
    streams 1 col/cycle at 2.4 GHz and FWL halves LDWEIGHTS time; fp32
    paths measured ~2x slower on HW. PSUM accumulation stays fp32.
  - q, k are produced TRANSPOSED ([head_cols, S]) so score matmuls need
    no on-device transposes; scores are computed transposed ([sk, sq])
    so the P @ v matmul consumes exp(scores) directly from SBUF.
  - v carries an appended ones-column per head; the attention output
    matmul then yields softmax row-sums in an extra partition row for
    free (no max-subtraction is needed: scores are O(5) so exp is safe
    in fp32, and masked entries are zeroed multiplicatively post-exp
    with a DVE multiply against a [128,128] triangle mask input).
  - Heads are processed in pairs: the two K=64 score matmuls sit in PE
    row-groups 0-63 / 64-127 and run concurrently in the array.
  - The P @ v matmuls run 2 chunks BEHIND the score/exp stream, so the
    in-order PE queue never stalls on the ACT engine's exp latency.
  - Inputs arrive in ~13 large DMAs (the Sync engine costs ~600ns per
    dma_start, so many small DMAs serialize the front); identity and
    triangle-mask constants are DMA inputs, so GPSIMD (with its ~6us
    first-call IRAM load) is never touched.
  - A warmup burst of identity matmuls runs during the input DMA so the
    HAM clock gate reaches 8/8 before the real stream starts; head
    finalization (transpose + normalize + store) is deferred into the
    following head-pair's stream, keeping the PE dense end-to-end.
"""
import sys

for _p in ("/opt/trn_rl_repo",):
    if _p not in sys.path:
        sys.path.insert(0, _p)

import numpy as np

B, S, HID = 2, 2048, 1024
NH, HD = 16, 64
NHL = 4            # heads per core
WC = NHL * HD      # 256 local q/k weight cols
VC = NHL * (HD + 1)  # 260 local v cols incl. ones col
NT = S // 128      # 16 key chunks
NA = S // 512      # 4 query stripes
NK = HID // 128    # 8 contraction chunks
LAG = 2            # P @ v trails the score/exp stream by this many chunks

_NC = None


def _build():
    from concourse import bacc, mybir
    from concourse.tile import TileContext

    FP = mybir.dt.float32
    BF = mybir.dt.bfloat16
    Exp = mybir.ActivationFunctionType.Exp

    nc = bacc.Bacc("TRN2", target_bir_lowering=False, debug=False, num_devices=8)

    xT = nc.dram_tensor("xT", [HID, S], BF, kind="ExternalInput")
    wq = nc.dram_tensor("wq", [HID, WC], BF, kind="ExternalInput")
    wk = nc.dram_tensor("wk", [HID, WC], BF, kind="ExternalInput")
    wv = nc.dram_tensor("wv", [HID + 1, VC], BF, kind="ExternalInput")
    bqk = nc.dram_tensor("bqk", [2 * WC, 1], FP, kind="ExternalInput")
    ident_d = nc.dram_tensor("ident", [128, 128], BF, kind="ExternalInput")
    tri_d = nc.dram_tensor("tri", [128, 128], BF, kind="ExternalInput")
    out = nc.dram_tensor("out", [S, WC], FP, kind="ExternalOutput")

    with TileContext(nc) as tc:
        with (
            tc.tile_pool(name="inp", bufs=1) as inp,
            tc.tile_pool(name="ptp", bufs=4) as ptp,
            tc.tile_pool(name="osb", bufs=8) as osb,
            tc.tile_pool(name="rcp", bufs=4) as rcp,
            tc.tile_pool(name="onat", bufs=4) as onp,
            tc.tile_pool(name="G", bufs=3, space="PSUM") as gp,
            tc.tile_pool(name="oT", bufs=2, space="PSUM") as otp,
        ):
            # PE warmup on a zeroed scratch tile (no DMA dependency, so it
            # starts right after the engine preambles): keeps the PE busy
            # through the HAM SHORT window while the input DMA streams, so
            # the projection stream starts at 2.4 GHz instead of 1.2
            scratch = inp.tile([128, 512], BF, name="scratch")
            nc.vector.memset(scratch[:, :], 0.0)
            warm = gp.tile([128, 1024], mybir.dt.float32, tag="G", name="warm")
            for _ in range(30):
                nc.tensor.matmul(warm[:, :512], lhsT=scratch[:, :128],
                                 rhs=scratch[:, :], start=True, stop=True)

            # ---- persistent inputs in SBUF, few large DMAs split across
            # the two HWDGE issue queues (Sync + Scalar). The front is
            # HBM-bandwidth-bound: issue the critical-path bytes (wq, x
            # quarter 0, wk, wv) before the remaining 3MB of x quarters,
            # which would otherwise steal bandwidth from them.
            ident = inp.tile([128, 128], BF, name="ident")
            nc.sync.dma_start(ident[:, :], ident_d[:, :])
            tri = inp.tile([128, 128], BF, name="tri")
            nc.scalar.dma_start(tri[:, :], tri_d[:, :])
            # wq/wk whole-tensor: [1024, 256] -> [128, 8*256] (k-major cols)
            wq_sb = inp.tile([128, NK * WC], BF, name="wq")
            nc.sync.dma_start(
                wq_sb[:, :].rearrange("p (k j) -> p k j", k=NK),
                wq[:, :].rearrange("(k p) j -> p k j", k=NK),
            )
            # v weights on the scalar queue: they gate stripe 0's out matmuls
            wv_sb = inp.tile([128, NK * VC], BF, name="wv")
            nc.scalar.dma_start(
                wv_sb[:, :].rearrange("p (k j) -> p k j", k=NK),
                wv[0:HID, :].rearrange("(k p) j -> p k j", k=NK),
            )
            # x quarter 0 in two halves (earlier first consumption)
            xq = [[None, None] for _ in range(4)]
            for h in range(2):
                t = inp.tile([128, 4 * 512], BF, name=f"xq0_{h}")
                nc.sync.dma_start(
                    t[:, :].rearrange("p (k j) -> p k j", k=4),
                    xT[h * 512:(h + 1) * 512, 0:512].rearrange(
                        "(k p) j -> p k j", k=4),
                )
                xq[0][h] = t
            wk_sb = inp.tile([128, NK * WC], BF, name="wk")
            nc.sync.dma_start(
                wk_sb[:, :].rearrange("p (k j) -> p k j", k=NK),
                wk[:, :].rearrange("(k p) j -> p k j", k=NK),
            )
            wv_last = inp.tile([1, VC], BF, name="wvl")
            nc.scalar.dma_start(wv_last[:, :], wv[HID:HID + 1, :])
            bqk_sb = inp.tile([128, 4], FP, name="bqk")
            nc.scalar.dma_start(
                bqk_sb[:, :].rearrange("p (t j) -> p t j", j=1),
                bqk[:, :].rearrange("(t p) j -> p t j", p=128),
            )
            for qtr, eng in ((1, nc.scalar), (2, nc.sync), (3, nc.sync)):
                t = inp.tile([128, NK * 512], BF, name=f"xq{qtr}")
                eng.dma_start(
                    t[:, :].rearrange("p (k j) -> p k j", k=NK),
                    xT[:, qtr * 512:(qtr + 1) * 512].rearrange(
                        "(k p) j -> p k j", k=NK),
                )
                xq[qtr][0] = t

            def xk(k, qtr):
                if qtr == 0:
                    t = xq[0][k // 4]
                    return t[:, (k % 4) * 512:(k % 4 + 1) * 512]
                return xq[qtr][0][:, k * 512:(k + 1) * 512]

            # split by S-quarter so interleaved later-quarter projection
            # writes can't false-depend against earlier attention reads
            qT_sb = [[inp.tile([128, 512], BF, name=f"qT{t}_{n}")
                      for n in range(4)] for t in range(2)]
            kT_sb = [[inp.tile([128, 512], BF, name=f"kT{t}_{n}")
                      for n in range(4)] for t in range(2)]
            v_sb = [inp.tile([128, VC], BF, name=f"v{c}") for c in range(NT)]

            # ---- projection emitters ----
            def proj_qk_unit(wt, bcol, dst, t, qtr):
                g = gp.tile([128, 1024], mybir.dt.float32, tag="G", name="g")
                for k in range(NK):
                    nc.tensor.matmul(
                        g[:, :512],
                        lhsT=wt[:, k * WC + t * 128:k * WC + (t + 1) * 128],
                        rhs=xk(k, qtr),
                        start=(k == 0), stop=(k == NK - 1),
                    )
                nc.vector.tensor_scalar_add(
                    dst[t][qtr][:, :], g[:, :512], bqk_sb[:, bcol + t:bcol + t + 1]
                )

            def proj_v_unit(c):
                qtr, cc = divmod(c, 4)
                g = gp.tile([128, 1024], mybir.dt.float32, tag="G", name="g")
                for k in range(NK):
                    nc.tensor.matmul(
                        g[:, :VC],
                        lhsT=xk(k, qtr)[:, cc * 128:(cc + 1) * 128],
                        rhs=wv_sb[:, k * VC:(k + 1) * VC],
                        start=(k == 0), stop=False,
                    )
                nc.tensor.matmul(  # bias row + ones column (K=1)
                    # tri row 0 is all-ones: broadcasts wv_last to all rows
                    g[:, :VC], lhsT=tri[0:1, 0:128], rhs=wv_last[:, :],
                    start=False, stop=True,
                )
                nc.vector.tensor_copy(v_sb[c][:, :], g[:, :VC])

            # ---- attention emitters ----
            # score/exp for ONE key chunk b of a head PAIR:
            # g = [h0-slice | h1-slice], one exp covers both heads
            def score_exp_unit(a, ht, b):
                g = gp.tile([128, 1024], mybir.dt.float32, tag="G", name="g")
                kn, ko = divmod(b * 128, 512)
                # diagonal chunks: columns < off are fully masked -- skip
                # them in the score matmul, the exp, and the P @ v matmul
                off = max(0, (b - 4 * a) * 128)
                for hh in range(2):
                    hb = hh * 64
                    nc.tensor.matmul(
                        g[:, hh * 512 + off:(hh + 1) * 512],
                        lhsT=kT_sb[ht][kn][hb:hb + 64, ko:ko + 128],
                        rhs=qT_sb[ht][a][hb:hb + 64, off:],
                        start=True, stop=True,
                    )
                pt = ptp.tile([128, 1024], BF, tag="pt", name="pt")
                if off:
                    gv = g[:, :].rearrange("p (h w) -> p h w", h=2)[:, :, off:]
                    pv = pt[:, :].rearrange("p (h w) -> p h w", h=2)[:, :, off:]
                    nc.scalar.activation(pv, gv, Exp, scale=HD ** -0.5)
                else:
                    nc.scalar.activation(pt[:, :], g[:, :], Exp, scale=HD ** -0.5)
                if b >= 4 * a:
                    # triangular boundary block: multiplicative mask on DVE
                    for hh in range(2):
                        c0 = hh * 512 + off
                        nc.vector.tensor_mul(
                            pt[:, c0:c0 + 128], pt[:, c0:c0 + 128], tri[:, :]
                        )
                return pt

            def av_unit(a, ht, b, nchunks, oTs, pt):
                off = max(0, (b - 4 * a) * 128)
                for hh in range(2):
                    h = 2 * ht + hh
                    nc.tensor.matmul(
                        oTs[hh][:, off:],
                        lhsT=v_sb[b][:, h * 65:(h + 1) * 65],
                        rhs=pt[:, hh * 512 + off:(hh + 1) * 512],
                        start=(b == 0), stop=(b == nchunks - 1),
                    )

            def finish_piece(a, ht, hh, c, oT_sb):
                # one 128-query block of one head: transpose + normalize.
                # single-transpose granularity spreads the (HAM-invisible)
                # PE transposes thinly through the stream
                h = 2 * ht + hh
                onat = onat_by_a[a]
                tr = gp.tile([128, HD + 1], BF, tag="G", name="tr")
                nc.tensor.transpose(
                    tr[:, :HD + 1], oT_sb[:, c * 128:(c + 1) * 128],
                    ident[:HD + 1, :HD + 1],
                )
                recip = rcp.tile([128, 1], FP, tag="recip", name="recip")
                nc.vector.reciprocal(recip[:, :], tr[:, HD:HD + 1])
                nc.vector.tensor_scalar_mul(
                    onat[:, c * WC + h * 64:c * WC + (h + 1) * 64],
                    tr[:, :HD], recip[:, :]
                )

            # ---- phase 1: the minimum needed by stripe a=0 head pair 0 ----
            proj_qk_unit(wq_sb, 0, qT_sb, 0, 0)
            proj_qk_unit(wk_sb, 2, kT_sb, 0, 0)
            proj_v_unit(0)
            proj_v_unit(1)

            # remaining projection units are doled out between attention
            # units, scheduled (just) before their first consumer, keeping
            # the PE busy while ACT works through the exp stream
            def q_(t, qtr):
                return lambda: proj_qk_unit(wq_sb, 0, qT_sb, t, qtr)

            def k_(t, qtr):
                return lambda: proj_qk_unit(wk_sb, 2, kT_sb, t, qtr)

            def v_(c):
                return lambda: proj_v_unit(c)

            # placement: just-before-first-consumer deadlines, spread so
            # every region keeps the PE slightly ahead of the exp stream
            # (pair tails especially: the flush P @ v matmuls wait there)
            filler = {
                0: [v_(2)], 1: [v_(3)], 2: [q_(1, 0)], 3: [k_(1, 0)],
                4: [q_(0, 1)], 5: [k_(0, 1)], 6: [v_(4)], 7: [v_(5)],
                9: [v_(6)], 11: [v_(7)], 13: [q_(1, 1)], 15: [k_(1, 1)],
                17: [q_(0, 2)], 20: [k_(0, 2)], 22: [v_(8)], 23: [v_(9)],
                26: [v_(10)], 29: [v_(11)], 32: [q_(1, 2)], 35: [k_(1, 2)],
                39: [q_(0, 3)], 43: [k_(0, 3)], 50: [v_(12)], 53: [q_(1, 3)],
                54: [v_(13)], 57: [k_(1, 3)], 58: [v_(14)], 61: [v_(15)],
            }

            onat_by_a = {}
            deferred = []          # finish/store closures fed into the stream

            # ---- phases 2+3: attention, software-pipelined ----
            uidx = 0
            for a in range(NA):
                nchunks = 4 * a + 4
                if a not in onat_by_a:
                    onat_by_a[a] = onp.tile([128, 4 * WC], FP, tag="onat",
                                            name="onat")
                for ht in range(2):
                    oTs = [otp.tile([HD + 1, 512], mybir.dt.float32,
                                    tag="oT", name="oT") for _ in range(2)]
                    pend = []
                    for b in range(nchunks):
                        pend.append((b, score_exp_unit(a, ht, b)))
                        if len(pend) > LAG:
                            bb, pt = pend.pop(0)
                            av_unit(a, ht, bb, nchunks, oTs, pt)
                        for f in filler.get(uidx, ()):
                            f()
                        for _ in range(2 if len(deferred) > 6 else 1):
                            if deferred:
                                deferred.pop(0)()
                        uidx += 1
                    # the tail P @ v matmuls and the oT psum drain flow into
                    # the NEXT pair's units (via the priority end of the
                    # deferred queue): by then the exp stream has caught up,
                    # so the in-order PE queue never waits at pair boundaries
                    bb0, pt0 = pend.pop(0)
                    av_unit(a, ht, bb0, nchunks, oTs, pt0)

                    def tail_av(a_=a, ht_=ht, p_=tuple(pend), n_=nchunks,
                                o_=oTs):
                        for bb, pt in p_:
                            av_unit(a_, ht_, bb, n_, o_, pt)

                    def drain(a_=a, ht_=ht, o_=oTs):
                        for hh in range(2):
                            oT_sb = osb.tile([HD + 1, 512], BF, tag="oTsb",
                                             name="oTsb")
                            nc.vector.tensor_copy(oT_sb[:, :], o_[hh][:, :])
                            for c in range(4):
                                deferred.append(
                                    (lambda hh_=hh, c_=c, t_=oT_sb:
                                     finish_piece(a_, ht_, hh_, c_, t_)))

                        # store this head pair once its finishes have run
                        def store():
                            nc.sync.dma_start(
                                out[a_ * 512:(a_ + 1) * 512,
                                    ht_ * 128:(ht_ + 1) * 128].rearrange(
                                    "(c p) j -> p c j", p=128),
                                onat_by_a[a_][:, :].rearrange(
                                    "p (c j) -> p c j", c=4)[
                                    :, :, ht_ * 128:(ht_ + 1) * 128],
                            )
                        deferred.append(store)

                    deferred.insert(0, drain)
                    deferred.insert(0, tail_av)
            while deferred:
                deferred.pop(0)()

    nc.compile()
    return nc


def _get_nc():
    global _NC
    if _NC is None:
        _NC = _build()
    return _NC


def make_in_maps(hidden_states, Wqk, bqk, Wv, bv):
    from ml_dtypes import bfloat16

    x = np.asarray(hidden_states, dtype=np.float32)
    Wqk = np.asarray(Wqk, dtype=np.float32)
    bqk = np.asarray(bqk, dtype=np.float32)
    Wv = np.asarray(Wv, dtype=np.float32)
    bv = np.asarray(bv, dtype=np.float32)

    ident = np.eye(128, dtype=bfloat16)
    tri = np.triu(np.ones((128, 128), np.float32)).astype(bfloat16)
    xTs = [np.ascontiguousarray(x[b].T.astype(bfloat16)) for b in range(B)]
    in_maps = []
    for c in range(8):
        b, ho = c // 4, (c % 4) * NHL
        cols = slice(ho * HD, (ho + NHL) * HD)
        wv_aug = np.zeros((HID + 1, VC), np.float32)
        for h in range(NHL):
            wv_aug[:HID, h * 65:h * 65 + HD] = Wv[:, (ho + h) * HD:(ho + h + 1) * HD]
            wv_aug[HID, h * 65:h * 65 + HD] = bv[(ho + h) * HD:(ho + h + 1) * HD]
            wv_aug[HID, h * 65 + HD] = 1.0
        bqk_c = np.concatenate([bqk[:HID][cols], bqk[HID:][cols]])
        in_maps.append({
            "xT": xTs[b],
            "wq": np.ascontiguousarray(Wqk[:, cols].astype(bfloat16)),
            "wk": np.ascontiguousarray(Wqk[:, HID:][:, cols].astype(bfloat16)),
            "wv": wv_aug.astype(bfloat16),
            "bqk": np.ascontiguousarray(bqk_c.reshape(2 * WC, 1)),
            "ident": ident,
            "tri": tri,
        })
    return in_maps


def kernel(hidden_states, Wqk, bqk, Wv, bv):
    import time

    from concourse.bass_utils import run_bass_kernel_spmd

    in_maps = make_in_maps(hidden_states, Wqk, bqk, Wv, bv)
    res = None
    for attempt in range(3):
        try:
            res = run_bass_kernel_spmd(_get_nc(), in_maps, list(range(8)))
            break
        except Exception:
            # transient NRT_EXEC_UNIT_UNRECOVERABLE errors have been observed
            # on this fabric; back off and retry
            if attempt == 2:
                raise
            time.sleep(2.0)
    outp = np.empty((B, S, NH * HD), np.float32)
    for c in range(8):
        b, ho = c // 4, (c % 4) * NHL
        outp[b, :, ho * HD:(ho + NHL) * HD] = res.results[c]["out"]
    return outp


# revision 18
# speedup vs baseline: 1.3826x; 1.0089x over previous
"""Causal self-attention (B=2, S=2048, HID=1024, 16 heads x 64) on 8 trn2
NeuronCores.

Sharding: data-parallel over batch (cores 0-3 -> batch 0, cores 4-7 ->
batch 1), tensor-parallel over heads (4 heads per core via Wqk/Wv column
slices). Each core computes its 4 heads end-to-end; the [S, S] score
matrix stays core-local.

Per-core layout choices:
  - All matmul operands are bf16 (inputs are cast host-side): the PE
    streams 1 col/cycle at 2.4 GHz and FWL halves LDWEIGHTS time; fp32
    paths measured ~2x slower on HW. PSUM accumulation stays fp32.
  - q, k are produced TRANSPOSED ([head_cols, S]) so score matmuls need
    no on-device transposes; scores are computed transposed ([sk, sq])
    so the P @ v matmul consumes exp(scores) directly from SBUF.
  - v carries an appended ones-column per head; the attention output
    matmul then yields softmax row-sums in an extra partition row for
    free (no max-subtraction is needed: scores are O(5) so exp is safe
    in fp32, and masked entries are zeroed multiplicatively post-exp
    with a DVE multiply against a [128,128] triangle mask input).
  - Heads are processed in pairs: the two K=64 score matmuls sit in PE
    row-groups 0-63 / 64-127 and run concurrently in the array.
  - The P @ v matmuls run 2 chunks BEHIND the score/exp stream, so the
    in-order PE queue never stalls on the ACT engine's exp latency.
  - Inputs arrive in ~13 large DMAs (the Sync engine costs ~600ns per
    dma_start, so many small DMAs serialize the front); identity and
    triangle-mask constants are DMA inputs, so GPSIMD (with its ~6us
    first-call IRAM load) is never touched.
  - A warmup burst of identity matmuls runs during the input DMA so the
    HAM clock gate reaches 8/8 before the real stream starts; head
    finalization (transpose + normalize + store) is deferred into the
    following head-pair's stream, keeping the PE dense end-to-end.
"""
import sys

for _p in ("/opt/trn_rl_repo",):
    if _p not in sys.path:
        sys.path.insert(0, _p)

import numpy as np

B, S, HID = 2, 2048, 1024
NH, HD = 16, 64
NHL = 4            # heads per core
WC = NHL * HD      # 256 local q/k weight cols
VC = NHL * (HD + 1)  # 260 local v cols incl. ones col
NT = S // 128      # 16 key chunks
NA = S // 512      # 4 query stripes
NK = HID // 128    # 8 contraction chunks
LAG = 2            # P @ v trails the score/exp stream by this many chunks

_NC = None


def _build():
    from concourse import bacc, mybir
    from concourse.tile import TileContext

    FP = mybir.dt.float32
    BF = mybir.dt.bfloat16
    Exp = mybir.ActivationFunctionType.Exp

    nc = bacc.Bacc("TRN2", target_bir_lowering=False, debug=False, num_devices=8)

    xT = nc.dram_tensor("xT", [HID, S], BF, kind="ExternalInput")
    wq = nc.dram_tensor("wq", [HID, WC], BF, kind="ExternalInput")
    wk = nc.dram_tensor("wk", [HID, WC], BF, kind="ExternalInput")
    wv = nc.dram_tensor("wv", [HID + 1, VC], BF, kind="ExternalInput")
    bqk = nc.dram_tensor("bqk", [2 * WC, 1], FP, kind="ExternalInput")
    ident_d = nc.dram_tensor("ident", [128, 128], BF, kind="ExternalInput")
    tri_d = nc.dram_tensor("tri", [128, 128], BF, kind="ExternalInput")
    out = nc.dram_tensor("out", [S, WC], FP, kind="ExternalOutput")

    with TileContext(nc) as tc:
        with (
            tc.tile_pool(name="inp", bufs=1) as inp,
            tc.tile_pool(name="ptp", bufs=4) as ptp,
            tc.tile_pool(name="osb", bufs=8) as osb,
            tc.tile_pool(name="rcp", bufs=4) as rcp,
            tc.tile_pool(name="onat", bufs=4) as onp,
            tc.tile_pool(name="G", bufs=3, space="PSUM") as gp,
            tc.tile_pool(name="oT", bufs=2, space="PSUM") as otp,
        ):
            # PE warmup on a zeroed scratch tile (no DMA dependency, so it
            # starts right after the engine preambles): keeps the PE busy
            # through the HAM SHORT window while the input DMA streams, so
            # the projection stream starts at 2.4 GHz instead of 1.2
            scratch = inp.tile([128, 512], BF, name="scratch")
            nc.vector.memset(scratch[:, :], 0.0)
            # preload the GPSIMD ucode IRAM (~6us, hidden in the preamble)
            # so the first in-stream gpsimd mask-multiply doesn't pay it
            gsc = inp.tile([128, 1], BF, name="gsc")
            nc.gpsimd.memset(gsc[:, :], 0.0)
            warm = gp.tile([128, 1024], mybir.dt.float32, tag="G", name="warm")
            for _ in range(44):
                nc.tensor.matmul(warm[:, :512], lhsT=scratch[:, :128],
                                 rhs=scratch[:, :], start=True, stop=True)

            # ---- persistent inputs in SBUF, few large DMAs split across
            # the two HWDGE issue queues (Sync + Scalar). The front is
            # HBM-bandwidth-bound: issue the critical-path bytes (wq, x
            # quarter 0, wk, wv) before the remaining 3MB of x quarters,
            # which would otherwise steal bandwidth from them.
            ident = inp.tile([128, 128], BF, name="ident")
            nc.sync.dma_start(ident[:, :], ident_d[:, :])
            tri = inp.tile([128, 128], BF, name="tri")
            nc.scalar.dma_start(tri[:, :], tri_d[:, :])
            # wq/wk whole-tensor: [1024, 256] -> [128, 8*256] (k-major cols)
            wq_sb = inp.tile([128, NK * WC], BF, name="wq")
            nc.sync.dma_start(
                wq_sb[:, :].rearrange("p (k j) -> p k j", k=NK),
                wq[:, :].rearrange("(k p) j -> p k j", k=NK),
            )
            # v weights on the scalar queue: they gate stripe 0's out matmuls
            wv_sb = inp.tile([128, NK * VC], BF, name="wv")
            nc.scalar.dma_start(
                wv_sb[:, :].rearrange("p (k j) -> p k j", k=NK),
                wv[0:HID, :].rearrange("(k p) j -> p k j", k=NK),
            )
            # x quarter 0 in two halves (earlier first consumption)
            xq = [[None, None] for _ in range(4)]
            for h in range(2):
                t = inp.tile([128, 4 * 512], BF, name=f"xq0_{h}")
                nc.sync.dma_start(
                    t[:, :].rearrange("p (k j) -> p k j", k=4),
                    xT[h * 512:(h + 1) * 512, 0:512].rearrange(
                        "(k p) j -> p k j", k=4),
                )
                xq[0][h] = t
            wk_sb = inp.tile([128, NK * WC], BF, name="wk")
            nc.sync.dma_start(
                wk_sb[:, :].rearrange("p (k j) -> p k j", k=NK),
                wk[:, :].rearrange("(k p) j -> p k j", k=NK),
            )
            wv_last = inp.tile([1, VC], BF, name="wvl")
            nc.scalar.dma_start(wv_last[:, :], wv[HID:HID + 1, :])
            bqk_sb = inp.tile([128, 4], FP, name="bqk")
            nc.scalar.dma_start(
                bqk_sb[:, :].rearrange("p (t j) -> p t j", j=1),
                bqk[:, :].rearrange("(t p) j -> p t j", p=128),
            )
            for qtr, eng in ((1, nc.scalar), (2, nc.sync), (3, nc.sync)):
                t = inp.tile([128, NK * 512], BF, name=f"xq{qtr}")
                eng.dma_start(
                    t[:, :].rearrange("p (k j) -> p k j", k=NK),
                    xT[:, qtr * 512:(qtr + 1) * 512].rearrange(
                        "(k p) j -> p k j", k=NK),
                )
                xq[qtr][0] = t

            def xk(k, qtr):
                if qtr == 0:
                    t = xq[0][k // 4]
                    return t[:, (k % 4) * 512:(k % 4 + 1) * 512]
                return xq[qtr][0][:, k * 512:(k + 1) * 512]

            # split by S-quarter so interleaved later-quarter projection
            # writes can't false-depend against earlier attention reads
            qT_sb = [[inp.tile([128, 512], BF, name=f"qT{t}_{n}")
                      for n in range(4)] for t in range(2)]
            kT_sb = [[inp.tile([128, 512], BF, name=f"kT{t}_{n}")
                      for n in range(4)] for t in range(2)]
            v_sb = [inp.tile([128, VC], BF, name=f"v{c}") for c in range(NT)]

            # ---- projection emitters ----
            def proj_qk_unit(wt, bcol, dst, t, qtr):
                g = gp.tile([128, 1024], mybir.dt.float32, tag="G", name="g")
                for k in range(NK):
                    nc.tensor.matmul(
                        g[:, :512],
                        lhsT=wt[:, k * WC + t * 128:k * WC + (t + 1) * 128],
                        rhs=xk(k, qtr),
                        start=(k == 0), stop=(k == NK - 1),
                    )
                nc.vector.tensor_scalar_add(
                    dst[t][qtr][:, :], g[:, :512], bqk_sb[:, bcol + t:bcol + t + 1]
                )

            def proj_v_unit(c):
                qtr, cc = divmod(c, 4)
                g = gp.tile([128, 1024], mybir.dt.float32, tag="G", name="g")
                for k in range(NK):
                    nc.tensor.matmul(
                        g[:, :VC],
                        lhsT=xk(k, qtr)[:, cc * 128:(cc + 1) * 128],
                        rhs=wv_sb[:, k * VC:(k + 1) * VC],
                        start=(k == 0), stop=False,
                    )
                nc.tensor.matmul(  # bias row + ones column (K=1)
                    # tri row 0 is all-ones: broadcasts wv_last to all rows
                    g[:, :VC], lhsT=tri[0:1, 0:128], rhs=wv_last[:, :],
                    start=False, stop=True,
                )
                nc.vector.tensor_copy(v_sb[c][:, :], g[:, :VC])

            # ---- attention emitters ----
            # score/exp for ONE key chunk b of a head PAIR:
            # g = [h0-slice | h1-slice], one exp covers both heads
            def score_exp_unit(a, ht, b):
                g = gp.tile([128, 1024], mybir.dt.float32, tag="G", name="g")
                kn, ko = divmod(b * 128, 512)
                # diagonal chunks: columns < off are fully masked -- skip
                # them in the score matmul, the exp, and the P @ v matmul
                off = max(0, (b - 4 * a) * 128)
                for hh in range(2):
                    hb = hh * 64
                    nc.tensor.matmul(
                        g[:, hh * 512 + off:(hh + 1) * 512],
                        lhsT=kT_sb[ht][kn][hb:hb + 64, ko:ko + 128],
                        rhs=qT_sb[ht][a][hb:hb + 64, off:],
                        start=True, stop=True,
                    )
                pt = ptp.tile([128, 1024], BF, tag="pt", name="pt")
                if off:
                    gv = g[:, :].rearrange("p (h w) -> p h w", h=2)[:, :, off:]
                    pv = pt[:, :].rearrange("p (h w) -> p h w", h=2)[:, :, off:]
                    nc.scalar.activation(pv, gv, Exp, scale=HD ** -0.5)
                else:
                    nc.scalar.activation(pt[:, :], g[:, :], Exp, scale=HD ** -0.5)
                if b >= 4 * a:
                    # triangular boundary block: multiplicative mask. On
                    # GPSIMD (otherwise idle) to keep the DVE queue short --
                    # DVE reads are what free PSUM slots for the PE.
                    for hh in range(2):
                        c0 = hh * 512 + off
                        nc.gpsimd.tensor_mul(
                            pt[:, c0:c0 + 128], pt[:, c0:c0 + 128], tri[:, :]
                        )
                return pt

            def av_unit(a, ht, b, nchunks, oTs, pt):
                off = max(0, (b - 4 * a) * 128)
                for hh in range(2):
                    h = 2 * ht + hh
                    nc.tensor.matmul(
                        oTs[hh][:, off:],
                        lhsT=v_sb[b][:, h * 65:(h + 1) * 65],
                        rhs=pt[:, hh * 512 + off:(hh + 1) * 512],
                        start=(b == 0), stop=(b == nchunks - 1),
                    )

            def finish_head(a, ht, hh, oT_sb):
                # transpose + normalize one head: all 4 query blocks go into
                # ONE psum tile, so the G ring is touched once per head (its
                # slot frees only when the DVE reads it -- fewer allocations
                # mean fewer PE stalls on the DVE queue)
                h = 2 * ht + hh
                onat = onat_by_a[a]
                # 66-wide slots keep each bf16 psum write 4-byte aligned
                tr = gp.tile([128, 4 * 66], BF, tag="G", name="tr")
                for c in range(4):
                    nc.tensor.transpose(
                        tr[:, c * 66:c * 66 + HD + 1],
                        oT_sb[:, c * 128:(c + 1) * 128],
                        ident[:HD + 1, :HD + 1],
                    )
                recip = rcp.tile([128, 4], FP, tag="recip", name="recip")
                trv = tr[:, :].rearrange("p (c d) -> p c d", c=4)
                nc.vector.reciprocal(recip[:, :], trv[:, :, HD])
                for c in range(4):
                    nc.vector.tensor_scalar_mul(
                        onat[:, c * WC + h * 64:c * WC + (h + 1) * 64],
                        tr[:, c * 66:c * 66 + HD], recip[:, c:c + 1]
                    )

            # ---- phase 1: the minimum needed by stripe a=0 head pair 0 ----
            proj_qk_unit(wq_sb, 0, qT_sb, 0, 0)
            proj_qk_unit(wk_sb, 2, kT_sb, 0, 0)
            proj_v_unit(0)
            proj_v_unit(1)

            # remaining projection units are doled out between attention
            # units, scheduled (just) before their first consumer, keeping
            # the PE busy while ACT works through the exp stream
            def q_(t, qtr):
                return lambda: proj_qk_unit(wq_sb, 0, qT_sb, t, qtr)

            def k_(t, qtr):
                return lambda: proj_qk_unit(wk_sb, 2, kT_sb, t, qtr)

            def v_(c):
                return lambda: proj_v_unit(c)

            # placement: just-before-first-consumer deadlines, spread so
            # every region keeps the PE slightly ahead of the exp stream
            # (pair tails especially: the flush P @ v matmuls wait there)
            filler = {
                0: [v_(2)], 1: [v_(3)], 2: [q_(1, 0)], 3: [k_(1, 0)],
                4: [q_(0, 1)], 5: [k_(0, 1)], 6: [v_(4)], 7: [v_(5)],
                9: [v_(6)], 11: [v_(7)], 13: [q_(1, 1)], 15: [k_(1, 1)],
                17: [q_(0, 2)], 20: [k_(0, 2)], 22: [v_(8)], 23: [v_(9)],
                26: [v_(10)], 29: [v_(11)], 32: [q_(1, 2)], 35: [k_(1, 2)],
                39: [q_(0, 3)], 43: [k_(0, 3)], 50: [v_(12)], 53: [q_(1, 3)],
                54: [v_(13)], 57: [k_(1, 3)], 58: [v_(14)], 61: [v_(15)],
            }

            onat_by_a = {}
            deferred = []          # finish/store closures fed into the stream

            # ---- phases 2+3: attention, software-pipelined ----
            uidx = 0
            for a in range(NA):
                nchunks = 4 * a + 4
                if a not in onat_by_a:
                    onat_by_a[a] = onp.tile([128, 4 * WC], FP, tag="onat",
                                            name="onat")
                for ht in range(2):
                    oTs = [otp.tile([HD + 1, 512], mybir.dt.float32,
                                    tag="oT", name="oT") for _ in range(2)]
                    pend = []
                    for b in range(nchunks):
                        pend.append((b, score_exp_unit(a, ht, b)))
                        if len(pend) > LAG:
                            bb, pt = pend.pop(0)
                            av_unit(a, ht, bb, nchunks, oTs, pt)
                        for f in filler.get(uidx, ()):
                            f()
                        for _ in range(2 if len(deferred) > 6 else 1):
                            if deferred:
                                deferred.pop(0)()
                        uidx += 1
                    # the tail P @ v matmuls and the oT psum drain flow into
                    # the NEXT pair's units (via the priority end of the
                    # deferred queue): by then the exp stream has caught up,
                    # so the in-order PE queue never waits at pair boundaries
                    bb0, pt0 = pend.pop(0)
                    av_unit(a, ht, bb0, nchunks, oTs, pt0)

                    def tail_av(a_=a, ht_=ht, p_=tuple(pend), n_=nchunks,
                                o_=oTs):
                        for bb, pt in p_:
                            av_unit(a_, ht_, bb, n_, o_, pt)

                    def drain(a_=a, ht_=ht, o_=oTs):
                        for hh in range(2):
                            oT_sb = osb.tile([HD + 1, 512], BF, tag="oTsb",
                                             name="oTsb")
                            nc.vector.tensor_copy(oT_sb[:, :], o_[hh][:, :])
                            deferred.append(
                                (lambda hh_=hh, t_=oT_sb:
                                 finish_head(a_, ht_, hh_, t_)))

                        # store this head pair once its finishes have run
                        def store():
                            nc.sync.dma_start(
                                out[a_ * 512:(a_ + 1) * 512,
                                    ht_ * 128:(ht_ + 1) * 128].rearrange(
                                    "(c p) j -> p c j", p=128),
                                onat_by_a[a_][:, :].rearrange(
                                    "p (c j) -> p c j", c=4)[
                                    :, :, ht_ * 128:(ht_ + 1) * 128],
                            )
                        deferred.append(store)

                    deferred.insert(0, drain)
                    deferred.insert(0, tail_av)
            while deferred:
                deferred.pop(0)()

    nc.compile()
    return nc


def _get_nc():
    global _NC
    if _NC is None:
        _NC = _build()
    return _NC


def make_in_maps(hidden_states, Wqk, bqk, Wv, bv):
    from ml_dtypes import bfloat16

    x = np.asarray(hidden_states, dtype=np.float32)
    Wqk = np.asarray(Wqk, dtype=np.float32)
    bqk = np.asarray(bqk, dtype=np.float32)
    Wv = np.asarray(Wv, dtype=np.float32)
    bv = np.asarray(bv, dtype=np.float32)

    ident = np.eye(128, dtype=bfloat16)
    tri = np.triu(np.ones((128, 128), np.float32)).astype(bfloat16)
    xTs = [np.ascontiguousarray(x[b].T.astype(bfloat16)) for b in range(B)]
    in_maps = []
    for c in range(8):
        b, ho = c // 4, (c % 4) * NHL
        cols = slice(ho * HD, (ho + NHL) * HD)
        wv_aug = np.zeros((HID + 1, VC), np.float32)
        for h in range(NHL):
            wv_aug[:HID, h * 65:h * 65 + HD] = Wv[:, (ho + h) * HD:(ho + h + 1) * HD]
            wv_aug[HID, h * 65:h * 65 + HD] = bv[(ho + h) * HD:(ho + h + 1) * HD]
            wv_aug[HID, h * 65 + HD] = 1.0
        bqk_c = np.concatenate([bqk[:HID][cols], bqk[HID:][cols]])
        in_maps.append({
            "xT": xTs[b],
            "wq": np.ascontiguousarray(Wqk[:, cols].astype(bfloat16)),
            "wk": np.ascontiguousarray(Wqk[:, HID:][:, cols].astype(bfloat16)),
            "wv": wv_aug.astype(bfloat16),
            "bqk": np.ascontiguousarray(bqk_c.reshape(2 * WC, 1)),
            "ident": ident,
            "tri": tri,
        })
    return in_maps


def kernel(hidden_states, Wqk, bqk, Wv, bv):
    import time

    from concourse.bass_utils import run_bass_kernel_spmd

    in_maps = make_in_maps(hidden_states, Wqk, bqk, Wv, bv)
    res = None
    for attempt in range(3):
        try:
            res = run_bass_kernel_spmd(_get_nc(), in_maps, list(range(8)))
            break
        except Exception:
            # transient NRT_EXEC_UNIT_UNRECOVERABLE errors have been observed
            # on this fabric; back off and retry
            if attempt == 2:
                raise
            time.sleep(2.0)
    outp = np.empty((B, S, NH * HD), np.float32)
    for c in range(8):
        b, ho = c // 4, (c % 4) * NHL
        outp[b, :, ho * HD:(ho + NHL) * HD] = res.results[c]["out"]
    return outp


# revision 25
# speedup vs baseline: 1.4406x; 1.0420x over previous
"""Causal self-attention (B=2, S=2048, HID=1024, 16 heads x 64) on 8 trn2
NeuronCores.

Sharding: data-parallel over batch (cores 0-3 -> batch 0, cores 4-7 ->
batch 1), tensor-parallel over heads (4 heads per core via Wqk/Wv column
slices). Each core computes its 4 heads end-to-end; the [S, S] score
matrix stays core-local.

Per-core layout choices:
  - All matmul operands are bf16 (inputs are cast host-side): the PE
    streams 1 col/cycle at 2.4 GHz and FWL halves LDWEIGHTS time; fp32
    paths measured ~2x slower on HW. PSUM accumulation stays fp32.
  - q, k are produced TRANSPOSED ([head_cols, S]) so score matmuls need
    no on-device transposes; scores are computed transposed ([sk, sq])
    so the P @ v matmul consumes exp(scores) directly from SBUF.
  - v carries an appended ones-column per head; the attention output
    matmul then yields softmax row-sums in an extra partition row for
    free (no max-subtraction is needed: scores are O(5) so exp is safe
    in fp32, and masked entries are zeroed multiplicatively post-exp
    with a DVE multiply against a [128,128] triangle mask input).
  - Heads are processed in pairs: the two K=64 score matmuls sit in PE
    row-groups 0-63 / 64-127 and run concurrently in the array.
  - The P @ v matmuls run 2 chunks BEHIND the score/exp stream, so the
    in-order PE queue never stalls on the ACT engine's exp latency.
  - Inputs arrive in ~13 large DMAs (the Sync engine costs ~600ns per
    dma_start, so many small DMAs serialize the front); identity and
    triangle-mask constants are DMA inputs, so GPSIMD (with its ~6us
    first-call IRAM load) is never touched.
  - A warmup burst of identity matmuls runs during the input DMA so the
    HAM clock gate reaches 8/8 before the real stream starts; head
    finalization (transpose + normalize + store) is deferred into the
    following head-pair's stream, keeping the PE dense end-to-end.
"""
import sys

for _p in ("/opt/trn_rl_repo",):
    if _p not in sys.path:
        sys.path.insert(0, _p)

import numpy as np

B, S, HID = 2, 2048, 1024
NH, HD = 16, 64
NHL = 4            # heads per core
WC = NHL * HD      # 256 local q/k weight cols
VC = NHL * (HD + 1)  # 260 local v cols incl. ones col
NT = S // 128      # 16 key chunks
NA = S // 512      # 4 query stripes
NK = HID // 128    # 8 contraction chunks
LAG = 2            # P @ v trails the score/exp stream by this many chunks

_NC = None


def _build():
    from concourse import bacc, mybir
    from concourse.tile import TileContext

    FP = mybir.dt.float32
    BF = mybir.dt.bfloat16
    Exp = mybir.ActivationFunctionType.Exp

    nc = bacc.Bacc("TRN2", target_bir_lowering=False, debug=False, num_devices=8)

    # all inputs are host-prepacked into the exact SBUF image, so every
    # input DMA is a fully-contiguous identity copy (8KB/partition runs)
    xq_d = [nc.dram_tensor(f"xq{q}", [128, NK * 512], BF, kind="ExternalInput")
            for q in range(4)]
    wq = nc.dram_tensor("wq", [128, NK * WC], BF, kind="ExternalInput")
    wk = nc.dram_tensor("wk", [128, NK * WC], BF, kind="ExternalInput")
    wv = nc.dram_tensor("wv", [128, NK * VC], BF, kind="ExternalInput")
    wvl_d = nc.dram_tensor("wvl", [1, VC], BF, kind="ExternalInput")
    bqk = nc.dram_tensor("bqk", [128, 4], FP, kind="ExternalInput")
    ident_d = nc.dram_tensor("ident", [128, 128], BF, kind="ExternalInput")
    tri_d = nc.dram_tensor("tri", [128, 128], BF, kind="ExternalInput")
    out = nc.dram_tensor("out", [S, WC], FP, kind="ExternalOutput")

    with TileContext(nc) as tc:
        with (
            tc.tile_pool(name="inp", bufs=1) as inp,
            tc.tile_pool(name="ptp", bufs=4) as ptp,
            tc.tile_pool(name="osb", bufs=8) as osb,
            tc.tile_pool(name="rcp", bufs=4) as rcp,
            tc.tile_pool(name="onat", bufs=4) as onp,
            tc.tile_pool(name="G", bufs=3, space="PSUM") as gp,
            tc.tile_pool(name="oT", bufs=2, space="PSUM") as otp,
        ):
            # PE warmup on a zeroed scratch tile (no DMA dependency, so it
            # starts right after the engine preambles): keeps the PE busy
            # through the HAM SHORT window while the input DMA streams, so
            # the projection stream starts at 2.4 GHz instead of 1.2
            scratch = inp.tile([128, 512], BF, name="scratch")
            nc.vector.memset(scratch[:, :], 0.0)
            # preload the GPSIMD ucode IRAM (~6us, hidden in the preamble)
            # so the first in-stream gpsimd mask-multiply doesn't pay it
            gsc = inp.tile([128, 1], BF, name="gsc")
            nc.gpsimd.memset(gsc[:, :], 0.0)
            warm = gp.tile([128, 1024], mybir.dt.float32, tag="G", name="warm")
            for _ in range(32):
                nc.tensor.matmul(warm[:, :512], lhsT=scratch[:, :128],
                                 rhs=scratch[:, :], start=True, stop=True)

            # ---- persistent inputs in SBUF, contiguous identity DMAs
            # split across the two HWDGE issue queues (Sync + Scalar).
            # The front is HBM-bandwidth-bound: issue the critical-path
            # bytes (wq, x quarter 0, wk, wv) before the remaining x
            # quarters, which would otherwise steal bandwidth from them.
            ident = inp.tile([128, 128], BF, name="ident")
            nc.sync.dma_start(ident[:, :], ident_d[:, :])
            tri = inp.tile([128, 128], BF, name="tri")
            nc.scalar.dma_start(tri[:, :], tri_d[:, :])
            # x quarter 0 in two halves, one per issue queue (the front is
            # aggregate-HBM-bound: critical tiles first, big quarters last)
            xq = [[None, None] for _ in range(4)]
            wq_sb = inp.tile([128, NK * WC], BF, name="wq")
            nc.sync.dma_start(wq_sb[:, :], wq[:, :])
            xq[0][1] = inp.tile([128, 4 * 512], BF, name="xq0_1")
            nc.scalar.dma_start(xq[0][1][:, :], xq_d[0][:, 2048:4096])
            xq[0][0] = inp.tile([128, 4 * 512], BF, name="xq0_0")
            nc.sync.dma_start(xq[0][0][:, :], xq_d[0][:, 0:2048])
            wv_sb = inp.tile([128, NK * VC], BF, name="wv")
            nc.scalar.dma_start(wv_sb[:, :], wv[:, :])
            wk_sb = inp.tile([128, NK * WC], BF, name="wk")
            nc.sync.dma_start(wk_sb[:, :], wk[:, :])
            wv_last = inp.tile([1, VC], BF, name="wvl")
            nc.scalar.dma_start(wv_last[:, :], wvl_d[:, :])
            bqk_sb = inp.tile([128, 4], FP, name="bqk")
            nc.scalar.dma_start(bqk_sb[:, :], bqk[:, :])
            for qtr, eng in ((1, nc.scalar), (2, nc.sync), (3, nc.sync)):
                t = inp.tile([128, NK * 512], BF, name=f"xq{qtr}")
                eng.dma_start(t[:, :], xq_d[qtr][:, :])
                xq[qtr][0] = t

            def xk(k, qtr):
                if qtr == 0:
                    t = xq[0][k // 4]
                    return t[:, (k % 4) * 512:(k % 4 + 1) * 512]
                return xq[qtr][0][:, k * 512:(k + 1) * 512]

            # split by S-quarter so interleaved later-quarter projection
            # writes can't false-depend against earlier attention reads
            qT_sb = [[inp.tile([128, 512], BF, name=f"qT{t}_{n}")
                      for n in range(4)] for t in range(2)]
            kT_sb = [[inp.tile([128, 512], BF, name=f"kT{t}_{n}")
                      for n in range(4)] for t in range(2)]
            v_sb = [inp.tile([128, VC], BF, name=f"v{c}") for c in range(NT)]

            # ---- projection emitters ----
            def proj_qk_unit(wt, bcol, dst, t, qtr):
                g = gp.tile([128, 1024], mybir.dt.float32, tag="G", name="g")
                for k in range(NK):
                    nc.tensor.matmul(
                        g[:, :512],
                        lhsT=wt[:, k * WC + t * 128:k * WC + (t + 1) * 128],
                        rhs=xk(k, qtr),
                        start=(k == 0), stop=(k == NK - 1),
                    )
                nc.vector.tensor_scalar_add(
                    dst[t][qtr][:, :], g[:, :512], bqk_sb[:, bcol + t:bcol + t + 1]
                )

            def proj_v_unit(c):
                qtr, cc = divmod(c, 4)
                g = gp.tile([128, 1024], mybir.dt.float32, tag="G", name="g")
                for k in range(NK):
                    nc.tensor.matmul(
                        g[:, :VC],
                        lhsT=xk(k, qtr)[:, cc * 128:(cc + 1) * 128],
                        rhs=wv_sb[:, k * VC:(k + 1) * VC],
                        start=(k == 0), stop=False,
                    )
                nc.tensor.matmul(  # bias row + ones column (K=1)
                    # tri row 0 is all-ones: broadcasts wv_last to all rows
                    g[:, :VC], lhsT=tri[0:1, 0:128], rhs=wv_last[:, :],
                    start=False, stop=True,
                )
                nc.vector.tensor_copy(v_sb[c][:, :], g[:, :VC])

            # ---- attention emitters ----
            # score/exp for ONE key chunk b of a head PAIR:
            # g = [h0-slice | h1-slice], one exp covers both heads
            def score_exp_unit(a, ht, b):
                g = gp.tile([128, 1024], mybir.dt.float32, tag="G", name="g")
                kn, ko = divmod(b * 128, 512)
                # diagonal chunks: columns < off are fully masked -- skip
                # them in the score matmul, the exp, and the P @ v matmul
                off = max(0, (b - 4 * a) * 128)
                for hh in range(2):
                    hb = hh * 64
                    nc.tensor.matmul(
                        g[:, hh * 512 + off:(hh + 1) * 512],
                        lhsT=kT_sb[ht][kn][hb:hb + 64, ko:ko + 128],
                        rhs=qT_sb[ht][a][hb:hb + 64, off:],
                        start=True, stop=True,
                    )
                pt = ptp.tile([128, 1024], BF, tag="pt", name="pt")
                if off:
                    gv = g[:, :].rearrange("p (h w) -> p h w", h=2)[:, :, off:]
                    pv = pt[:, :].rearrange("p (h w) -> p h w", h=2)[:, :, off:]
                    nc.scalar.activation(pv, gv, Exp, scale=HD ** -0.5)
                else:
                    nc.scalar.activation(pt[:, :], g[:, :], Exp, scale=HD ** -0.5)
                if b >= 4 * a:
                    # triangular boundary block: multiplicative mask. On
                    # GPSIMD (otherwise idle) to keep the DVE queue short --
                    # DVE reads are what free PSUM slots for the PE.
                    for hh in range(2):
                        c0 = hh * 512 + off
                        nc.gpsimd.tensor_mul(
                            pt[:, c0:c0 + 128], pt[:, c0:c0 + 128], tri[:, :]
                        )
                return pt

            def av_unit(a, ht, b, nchunks, oTs, pt):
                off = max(0, (b - 4 * a) * 128)
                for hh in range(2):
                    h = 2 * ht + hh
                    nc.tensor.matmul(
                        oTs[hh][:, off:],
                        lhsT=v_sb[b][:, h * 65:(h + 1) * 65],
                        rhs=pt[:, hh * 512 + off:(hh + 1) * 512],
                        start=(b == 0), stop=(b == nchunks - 1),
                    )

            def finish_head(a, ht, hh, oT_sb):
                # transpose + normalize one head: all 4 query blocks go into
                # ONE psum tile, so the G ring is touched once per head (its
                # slot frees only when the DVE reads it -- fewer allocations
                # mean fewer PE stalls on the DVE queue)
                h = 2 * ht + hh
                onat = onat_by_a[a]
                # 66-wide slots keep each bf16 psum write 4-byte aligned
                tr = gp.tile([128, 4 * 66], BF, tag="G", name="tr")
                for c in range(4):
                    nc.tensor.transpose(
                        tr[:, c * 66:c * 66 + HD + 1],
                        oT_sb[:, c * 128:(c + 1) * 128],
                        ident[:HD + 1, :HD + 1],
                    )
                recip = rcp.tile([128, 4], FP, tag="recip", name="recip")
                trv = tr[:, :].rearrange("p (c d) -> p c d", c=4)
                nc.vector.reciprocal(recip[:, :], trv[:, :, HD])
                for c in range(4):
                    nc.vector.tensor_scalar_mul(
                        onat[:, c * WC + h * 64:c * WC + (h + 1) * 64],
                        tr[:, c * 66:c * 66 + HD], recip[:, c:c + 1]
                    )

            # ---- phase 1: the minimum needed by stripe a=0 head pair 0 ----
            proj_qk_unit(wq_sb, 0, qT_sb, 0, 0)
            proj_qk_unit(wk_sb, 2, kT_sb, 0, 0)
            proj_v_unit(0)
            proj_v_unit(1)

            # remaining projection units are doled out between attention
            # units, scheduled (just) before their first consumer, keeping
            # the PE busy while ACT works through the exp stream
            def q_(t, qtr):
                return lambda: proj_qk_unit(wq_sb, 0, qT_sb, t, qtr)

            def k_(t, qtr):
                return lambda: proj_qk_unit(wk_sb, 2, kT_sb, t, qtr)

            def v_(c):
                return lambda: proj_v_unit(c)

            # placement: just-before-first-consumer deadlines, spread so
            # every region keeps the PE slightly ahead of the exp stream.
            # Stripes run 0,1,3,2: the trailing stripe-2 region (24 units)
            # then still has its own q/k projections left as PE filler,
            # where stripe 3 last would leave the PE starved (and the HAM
            # clock gate re-throttling) for its final 16 units.
            # NOTE: stripe 3 consumes ALL kT quarters and v chunks, so only
            # the stripe-2 q projections can be held back for the tail
            filler = {
                0: [v_(2)], 1: [v_(3)], 2: [q_(1, 0)], 3: [k_(1, 0)],
                4: [q_(0, 1)], 5: [k_(0, 1)], 6: [v_(4)], 7: [v_(5)],
                9: [v_(6)], 11: [v_(7)], 13: [q_(1, 1)], 15: [k_(1, 1)],
                17: [q_(0, 3)], 19: [k_(0, 3)], 21: [v_(8)], 23: [k_(0, 2)],
                25: [v_(9)], 26: [v_(10)], 28: [v_(11)], 30: [v_(12)],
                32: [v_(13)], 34: [v_(14)], 36: [v_(15)], 38: [q_(1, 3)],
                39: [k_(1, 3)], 44: [k_(1, 2)], 50: [q_(0, 2)],
                58: [q_(1, 2)],
            }

            onat_by_a = {}
            deferred = []          # finish/store closures fed into the stream

            # ---- phases 2+3: attention, software-pipelined ----
            uidx = 0
            for a in (0, 1, 3, 2):
                nchunks = 4 * a + 4
                if a not in onat_by_a:
                    onat_by_a[a] = onp.tile([128, 4 * WC], FP, tag="onat",
                                            name="onat")
                for ht in range(2):
                    oTs = [otp.tile([HD + 1, 512], mybir.dt.float32,
                                    tag="oT", name="oT") for _ in range(2)]
                    pend = []
                    for b in range(nchunks):
                        pend.append((b, score_exp_unit(a, ht, b)))
                        if len(pend) > LAG:
                            bb, pt = pend.pop(0)
                            av_unit(a, ht, bb, nchunks, oTs, pt)
                        for f in filler.get(uidx, ()):
                            f()
                        for _ in range(2 if len(deferred) > 6 else 1):
                            if deferred:
                                deferred.pop(0)()
                        uidx += 1
                    # the tail P @ v matmuls and the oT psum drain flow into
                    # the NEXT pair's units (via the priority end of the
                    # deferred queue): by then the exp stream has caught up,
                    # so the in-order PE queue never waits at pair boundaries
                    bb0, pt0 = pend.pop(0)
                    av_unit(a, ht, bb0, nchunks, oTs, pt0)

                    def tail_av(a_=a, ht_=ht, p_=tuple(pend), n_=nchunks,
                                o_=oTs):
                        for bb, pt in p_:
                            av_unit(a_, ht_, bb, n_, o_, pt)

                    def drain(a_=a, ht_=ht, o_=oTs):
                        for hh in range(2):
                            oT_sb = osb.tile([HD + 1, 512], BF, tag="oTsb",
                                             name="oTsb")
                            nc.vector.tensor_copy(oT_sb[:, :], o_[hh][:, :])
                            deferred.append(
                                (lambda hh_=hh, t_=oT_sb:
                                 finish_head(a_, ht_, hh_, t_)))

                        # store this head pair once its finishes have run
                        def store():
                            nc.sync.dma_start(
                                out[a_ * 512:(a_ + 1) * 512,
                                    ht_ * 128:(ht_ + 1) * 128].rearrange(
                                    "(c p) j -> p c j", p=128),
                                onat_by_a[a_][:, :].rearrange(
                                    "p (c j) -> p c j", c=4)[
                                    :, :, ht_ * 128:(ht_ + 1) * 128],
                            )
                        deferred.append(store)

                    deferred.insert(0, drain)
                    deferred.insert(0, tail_av)
            while deferred:
                deferred.pop(0)()

    nc.compile()
    return nc


def _get_nc():
    global _NC
    if _NC is None:
        _NC = _build()
    return _NC


def make_in_maps(hidden_states, Wqk, bqk, Wv, bv):
    from ml_dtypes import bfloat16

    x = np.asarray(hidden_states, dtype=np.float32)
    Wqk = np.asarray(Wqk, dtype=np.float32)
    bqk = np.asarray(bqk, dtype=np.float32)
    Wv = np.asarray(Wv, dtype=np.float32)
    bv = np.asarray(bv, dtype=np.float32)

    def pack(w):
        # [1024, C] -> SBUF image [128, 8*C] (k-chunk-major columns)
        c = w.shape[1]
        return np.ascontiguousarray(
            w.reshape(NK, 128, c).transpose(1, 0, 2).reshape(128, NK * c)
        ).astype(bfloat16)

    ident = np.eye(128, dtype=bfloat16)
    tri = np.triu(np.ones((128, 128), np.float32)).astype(bfloat16)
    # x quarters as SBUF images: xq[q][p, k*512+j] = x[b].T[k*128+p, q*512+j]
    xqs = []
    for b in range(B):
        xT = x[b].T.reshape(NK, 128, 4, 512)
        xqs.append([np.ascontiguousarray(
            xT[:, :, q, :].transpose(1, 0, 2).reshape(128, NK * 512)
        ).astype(bfloat16) for q in range(4)])
    in_maps = []
    for c in range(8):
        b, ho = c // 4, (c % 4) * NHL
        cols = slice(ho * HD, (ho + NHL) * HD)
        wv_aug = np.zeros((HID, VC), np.float32)
        wvl = np.zeros((1, VC), np.float32)
        for h in range(NHL):
            wv_aug[:, h * 65:h * 65 + HD] = Wv[:, (ho + h) * HD:(ho + h + 1) * HD]
            wvl[0, h * 65:h * 65 + HD] = bv[(ho + h) * HD:(ho + h + 1) * HD]
            wvl[0, h * 65 + HD] = 1.0
        bqk_c = np.stack([bqk[:HID][cols][:128], bqk[:HID][cols][128:],
                          bqk[HID:][cols][:128], bqk[HID:][cols][128:]],
                         axis=1)
        m = {
            "wq": pack(Wqk[:, cols]),
            "wk": pack(Wqk[:, HID:][:, cols]),
            "wv": pack(wv_aug),
            "wvl": wvl.astype(bfloat16),
            "bqk": np.ascontiguousarray(bqk_c.astype(np.float32)),
            "ident": ident,
            "tri": tri,
        }
        for q in range(4):
            m[f"xq{q}"] = xqs[b][q]
        in_maps.append(m)
    return in_maps


def kernel(hidden_states, Wqk, bqk, Wv, bv):
    import time

    from concourse.bass_utils import run_bass_kernel_spmd

    in_maps = make_in_maps(hidden_states, Wqk, bqk, Wv, bv)
    res = None
    for attempt in range(3):
        try:
            res = run_bass_kernel_spmd(_get_nc(), in_maps, list(range(8)))
            break
        except Exception:
            # transient NRT_EXEC_UNIT_UNRECOVERABLE errors have been observed
            # on this fabric; back off and retry
            if attempt == 2:
                raise
            time.sleep(2.0)
    outp = np.empty((B, S, NH * HD), np.float32)
    for c in range(8):
        b, ho = c // 4, (c % 4) * NHL
        outp[b, :, ho * HD:(ho + NHL) * HD] = res.results[c]["out"]
    return outp


# revision 26
# speedup vs baseline: 1.4511x; 1.0072x over previous
"""Causal self-attention (B=2, S=2048, HID=1024, 16 heads x 64) on 8 trn2
NeuronCores.

Sharding: data-parallel over batch (cores 0-3 -> batch 0, cores 4-7 ->
batch 1), tensor-parallel over heads (4 heads per core via Wqk/Wv column
slices). Each core computes its 4 heads end-to-end; the [S, S] score
matrix stays core-local.

Per-core layout choices:
  - All matmul operands are bf16 (inputs are cast host-side): the PE
    streams 1 col/cycle at 2.4 GHz and FWL halves LDWEIGHTS time; fp32
    paths measured ~2x slower on HW. PSUM accumulation stays fp32.
  - q, k are produced TRANSPOSED ([head_cols, S]) so score matmuls need
    no on-device transposes; scores are computed transposed ([sk, sq])
    so the P @ v matmul consumes exp(scores) directly from SBUF.
  - v carries an appended ones-column per head; the attention output
    matmul then yields softmax row-sums in an extra partition row for
    free (no max-subtraction is needed: scores are O(5) so exp is safe
    in fp32, and masked entries are zeroed multiplicatively post-exp
    with a DVE multiply against a [128,128] triangle mask input).
  - Heads are processed in pairs: the two K=64 score matmuls sit in PE
    row-groups 0-63 / 64-127 and run concurrently in the array.
  - The P @ v matmuls run 2 chunks BEHIND the score/exp stream, so the
    in-order PE queue never stalls on the ACT engine's exp latency.
  - Inputs arrive host-prepacked in the exact SBUF image as ~13 large
    contiguous DMAs split across the two HWDGE issue queues (Sync +
    Scalar; each dma_start costs ~0.6-1us of issue time, and the front
    is aggregate-HBM-bandwidth-bound, so critical tiles issue first).
  - The triangle masking runs on GPSIMD (IRAM preloaded at t=0), off
    the exp -> P @ v critical path, keeping the DVE queue short: DVE
    reads are what free PSUM tiles for the next score matmuls.
  - Stripes run 0,1,3,2 so the trailing region still has projection
    work as PE filler; pair-tail P @ v + PSUM drain + head finalization
    (transpose + normalize + store) are deferred into the following
    units via a priority queue. With a scratch-tile warmup burst
    bridging the input DMA, the PE holds the HAM clock gate at 8/8 from
    ~14us to the tail (measured: one continuous 123us warm window).
"""
import sys

for _p in ("/opt/trn_rl_repo",):
    if _p not in sys.path:
        sys.path.insert(0, _p)

import numpy as np

B, S, HID = 2, 2048, 1024
NH, HD = 16, 64
NHL = 4            # heads per core
WC = NHL * HD      # 256 local q/k weight cols
VC = NHL * (HD + 1)  # 260 local v cols incl. ones col
NT = S // 128      # 16 key chunks
NA = S // 512      # 4 query stripes
NK = HID // 128    # 8 contraction chunks
LAG = 2            # P @ v trails the score/exp stream by this many chunks

_NC = None


def _build():
    from concourse import bacc, mybir
    from concourse.tile import TileContext

    FP = mybir.dt.float32
    BF = mybir.dt.bfloat16
    Exp = mybir.ActivationFunctionType.Exp

    nc = bacc.Bacc("TRN2", target_bir_lowering=False, debug=False, num_devices=8)

    # all inputs are host-prepacked into the exact SBUF image, so every
    # input DMA is a fully-contiguous identity copy (8KB/partition runs)
    xq_d = [nc.dram_tensor(f"xq{q}", [128, NK * 512], BF, kind="ExternalInput")
            for q in range(4)]
    wq = nc.dram_tensor("wq", [128, NK * WC], BF, kind="ExternalInput")
    wk = nc.dram_tensor("wk", [128, NK * WC], BF, kind="ExternalInput")
    wv = nc.dram_tensor("wv", [128, NK * VC], BF, kind="ExternalInput")
    wvl_d = nc.dram_tensor("wvl", [1, VC], BF, kind="ExternalInput")
    bqk = nc.dram_tensor("bqk", [128, 4], FP, kind="ExternalInput")
    ident_d = nc.dram_tensor("ident", [128, 128], BF, kind="ExternalInput")
    tri_d = nc.dram_tensor("tri", [128, 128], BF, kind="ExternalInput")
    out = nc.dram_tensor("out", [S, WC], FP, kind="ExternalOutput")

    with TileContext(nc) as tc:
        with (
            tc.tile_pool(name="inp", bufs=1) as inp,
            tc.tile_pool(name="ptp", bufs=4) as ptp,
            tc.tile_pool(name="osb", bufs=8) as osb,
            tc.tile_pool(name="rcp", bufs=4) as rcp,
            tc.tile_pool(name="onat", bufs=4) as onp,
            tc.tile_pool(name="G", bufs=3, space="PSUM") as gp,
            tc.tile_pool(name="oT", bufs=2, space="PSUM") as otp,
        ):
            # PE warmup on a zeroed scratch tile (no DMA dependency, so it
            # starts right after the engine preambles): keeps the PE busy
            # through the HAM SHORT window while the input DMA streams, so
            # the projection stream starts at 2.4 GHz instead of 1.2
            scratch = inp.tile([128, 512], BF, name="scratch")
            nc.vector.memset(scratch[:, :], 0.0)
            # preload the GPSIMD ucode IRAM (~6us, hidden in the preamble)
            # so the first in-stream gpsimd mask-multiply doesn't pay it
            gsc = inp.tile([128, 1], BF, name="gsc")
            nc.gpsimd.memset(gsc[:, :], 0.0)
            warm = gp.tile([128, 1024], mybir.dt.float32, tag="G", name="warm")
            for _ in range(32):
                nc.tensor.matmul(warm[:, :512], lhsT=scratch[:, :128],
                                 rhs=scratch[:, :], start=True, stop=True)

            # ---- persistent inputs in SBUF, contiguous identity DMAs
            # split across the two HWDGE issue queues (Sync + Scalar).
            # The front is HBM-bandwidth-bound: issue the critical-path
            # bytes (wq, x quarter 0, wk, wv) before the remaining x
            # quarters, which would otherwise steal bandwidth from them.
            ident = inp.tile([128, 128], BF, name="ident")
            nc.sync.dma_start(ident[:, :], ident_d[:, :])
            tri = inp.tile([128, 128], BF, name="tri")
            nc.scalar.dma_start(tri[:, :], tri_d[:, :])
            # x quarter 0 in two halves, one per issue queue (the front is
            # aggregate-HBM-bound: critical tiles first, big quarters last)
            xq = [[None, None] for _ in range(4)]
            wq_sb = inp.tile([128, NK * WC], BF, name="wq")
            nc.sync.dma_start(wq_sb[:, :], wq[:, :])
            xq[0][1] = inp.tile([128, 4 * 512], BF, name="xq0_1")
            nc.scalar.dma_start(xq[0][1][:, :], xq_d[0][:, 2048:4096])
            xq[0][0] = inp.tile([128, 4 * 512], BF, name="xq0_0")
            nc.sync.dma_start(xq[0][0][:, :], xq_d[0][:, 0:2048])
            wv_sb = inp.tile([128, NK * VC], BF, name="wv")
            nc.scalar.dma_start(wv_sb[:, :], wv[:, :])
            wk_sb = inp.tile([128, NK * WC], BF, name="wk")
            nc.sync.dma_start(wk_sb[:, :], wk[:, :])
            wv_last = inp.tile([1, VC], BF, name="wvl")
            nc.scalar.dma_start(wv_last[:, :], wvl_d[:, :])
            bqk_sb = inp.tile([128, 4], FP, name="bqk")
            nc.scalar.dma_start(bqk_sb[:, :], bqk[:, :])
            for qtr, eng in ((1, nc.scalar), (2, nc.sync), (3, nc.sync)):
                t = inp.tile([128, NK * 512], BF, name=f"xq{qtr}")
                eng.dma_start(t[:, :], xq_d[qtr][:, :])
                xq[qtr][0] = t

            def xk(k, qtr):
                if qtr == 0:
                    t = xq[0][k // 4]
                    return t[:, (k % 4) * 512:(k % 4 + 1) * 512]
                return xq[qtr][0][:, k * 512:(k + 1) * 512]

            # split by S-quarter so interleaved later-quarter projection
            # writes can't false-depend against earlier attention reads
            qT_sb = [[inp.tile([128, 512], BF, name=f"qT{t}_{n}")
                      for n in range(4)] for t in range(2)]
            kT_sb = [[inp.tile([128, 512], BF, name=f"kT{t}_{n}")
                      for n in range(4)] for t in range(2)]
            v_sb = [inp.tile([128, VC], BF, name=f"v{c}") for c in range(NT)]

            # ---- projection emitters ----
            def proj_qk_unit(wt, bcol, dst, t, qtr):
                g = gp.tile([128, 1024], mybir.dt.float32, tag="G", name="g")
                for k in range(NK):
                    nc.tensor.matmul(
                        g[:, :512],
                        lhsT=wt[:, k * WC + t * 128:k * WC + (t + 1) * 128],
                        rhs=xk(k, qtr),
                        start=(k == 0), stop=(k == NK - 1),
                    )
                nc.vector.tensor_scalar_add(
                    dst[t][qtr][:, :], g[:, :512], bqk_sb[:, bcol + t:bcol + t + 1]
                )

            def proj_v_unit(c):
                qtr, cc = divmod(c, 4)
                g = gp.tile([128, 1024], mybir.dt.float32, tag="G", name="g")
                for k in range(NK):
                    nc.tensor.matmul(
                        g[:, :VC],
                        lhsT=xk(k, qtr)[:, cc * 128:(cc + 1) * 128],
                        rhs=wv_sb[:, k * VC:(k + 1) * VC],
                        start=(k == 0), stop=False,
                    )
                nc.tensor.matmul(  # bias row + ones column (K=1)
                    # tri row 0 is all-ones: broadcasts wv_last to all rows
                    g[:, :VC], lhsT=tri[0:1, 0:128], rhs=wv_last[:, :],
                    start=False, stop=True,
                )
                nc.vector.tensor_copy(v_sb[c][:, :], g[:, :VC])

            # ---- attention emitters ----
            # score/exp for ONE key chunk b of a head PAIR:
            # g = [h0-slice | h1-slice], one exp covers both heads
            def score_exp_unit(a, ht, b):
                g = gp.tile([128, 1024], mybir.dt.float32, tag="G", name="g")
                kn, ko = divmod(b * 128, 512)
                # diagonal chunks: columns < off are fully masked -- skip
                # them in the score matmul, the exp, and the P @ v matmul
                off = max(0, (b - 4 * a) * 128)
                for hh in range(2):
                    hb = hh * 64
                    nc.tensor.matmul(
                        g[:, hh * 512 + off:(hh + 1) * 512],
                        lhsT=kT_sb[ht][kn][hb:hb + 64, ko:ko + 128],
                        rhs=qT_sb[ht][a][hb:hb + 64, off:],
                        start=True, stop=True,
                    )
                pt = ptp.tile([128, 1024], BF, tag="pt", name="pt")
                if off:
                    gv = g[:, :].rearrange("p (h w) -> p h w", h=2)[:, :, off:]
                    pv = pt[:, :].rearrange("p (h w) -> p h w", h=2)[:, :, off:]
                    nc.scalar.activation(pv, gv, Exp, scale=HD ** -0.5)
                else:
                    nc.scalar.activation(pt[:, :], g[:, :], Exp, scale=HD ** -0.5)
                if b >= 4 * a:
                    # triangular boundary block: multiplicative mask. On
                    # GPSIMD (otherwise idle) to keep the DVE queue short --
                    # DVE reads are what free PSUM slots for the PE.
                    for hh in range(2):
                        c0 = hh * 512 + off
                        nc.gpsimd.tensor_mul(
                            pt[:, c0:c0 + 128], pt[:, c0:c0 + 128], tri[:, :]
                        )
                return pt

            def av_unit(a, ht, b, nchunks, oTs, pt):
                off = max(0, (b - 4 * a) * 128)
                for hh in range(2):
                    h = 2 * ht + hh
                    nc.tensor.matmul(
                        oTs[hh][:, off:],
                        lhsT=v_sb[b][:, h * 65:(h + 1) * 65],
                        rhs=pt[:, hh * 512 + off:(hh + 1) * 512],
                        start=(b == 0), stop=(b == nchunks - 1),
                    )

            def finish_head(a, ht, hh, oT_sb):
                # transpose + normalize one head: all 4 query blocks go into
                # ONE psum tile, so the G ring is touched once per head (its
                # slot frees only when the DVE reads it -- fewer allocations
                # mean fewer PE stalls on the DVE queue)
                h = 2 * ht + hh
                onat = onat_by_a[a]
                # 66-wide slots keep each bf16 psum write 4-byte aligned
                tr = gp.tile([128, 4 * 66], BF, tag="G", name="tr")
                for c in range(4):
                    nc.tensor.transpose(
                        tr[:, c * 66:c * 66 + HD + 1],
                        oT_sb[:, c * 128:(c + 1) * 128],
                        ident[:HD + 1, :HD + 1],
                    )
                recip = rcp.tile([128, 4], FP, tag="recip", name="recip")
                trv = tr[:, :].rearrange("p (c d) -> p c d", c=4)
                nc.vector.reciprocal(recip[:, :], trv[:, :, HD])
                for c in range(4):
                    nc.vector.tensor_scalar_mul(
                        onat[:, c * WC + h * 64:c * WC + (h + 1) * 64],
                        tr[:, c * 66:c * 66 + HD], recip[:, c:c + 1]
                    )

            # ---- phase 1: the minimum needed by stripe a=0 head pair 0 ----
            proj_qk_unit(wq_sb, 0, qT_sb, 0, 0)
            proj_qk_unit(wk_sb, 2, kT_sb, 0, 0)
            proj_v_unit(0)
            proj_v_unit(1)

            # remaining projection units are doled out between attention
            # units, scheduled (just) before their first consumer, keeping
            # the PE busy while ACT works through the exp stream
            def q_(t, qtr):
                return lambda: proj_qk_unit(wq_sb, 0, qT_sb, t, qtr)

            def k_(t, qtr):
                return lambda: proj_qk_unit(wk_sb, 2, kT_sb, t, qtr)

            def v_(c):
                return lambda: proj_v_unit(c)

            # placement: just-before-first-consumer deadlines, spread so
            # every region keeps the PE slightly ahead of the exp stream.
            # Stripes run 0,1,3,2: the trailing stripe-2 region (24 units)
            # then still has its own q/k projections left as PE filler,
            # where stripe 3 last would leave the PE starved (and the HAM
            # clock gate re-throttling) for its final 16 units.
            # NOTE: stripe 3 consumes ALL kT quarters and v chunks, so only
            # the stripe-2 q projections can be held back for the tail
            filler = {
                0: [v_(2)], 1: [v_(3)], 2: [q_(1, 0)], 3: [k_(1, 0)],
                4: [q_(0, 1)], 5: [k_(0, 1)], 6: [v_(4)], 7: [v_(5)],
                9: [v_(6)], 11: [v_(7)], 13: [q_(1, 1)], 15: [k_(1, 1)],
                17: [q_(0, 3)], 19: [k_(0, 3)], 21: [v_(8)], 23: [k_(0, 2)],
                25: [v_(9)], 26: [v_(10)], 28: [v_(11)], 30: [v_(12)],
                32: [v_(13)], 34: [v_(14)], 36: [v_(15)], 38: [q_(1, 3)],
                39: [k_(1, 3)], 44: [k_(1, 2)], 50: [q_(0, 2)],
                58: [q_(1, 2)],
            }

            onat_by_a = {}
            deferred = []          # finish/store closures fed into the stream

            # ---- phases 2+3: attention, software-pipelined ----
            uidx = 0
            for a in (0, 1, 3, 2):
                nchunks = 4 * a + 4
                if a not in onat_by_a:
                    onat_by_a[a] = onp.tile([128, 4 * WC], FP, tag="onat",
                                            name="onat")
                for ht in range(2):
                    oTs = [otp.tile([HD + 1, 512], mybir.dt.float32,
                                    tag="oT", name="oT") for _ in range(2)]
                    pend = []
                    for b in range(nchunks):
                        pend.append((b, score_exp_unit(a, ht, b)))
                        if len(pend) > LAG:
                            bb, pt = pend.pop(0)
                            av_unit(a, ht, bb, nchunks, oTs, pt)
                        for f in filler.get(uidx, ()):
                            f()
                        for _ in range(2 if len(deferred) > 6 else 1):
                            if deferred:
                                deferred.pop(0)()
                        uidx += 1
                    # the tail P @ v matmuls and the oT psum drain flow into
                    # the NEXT pair's units (via the priority end of the
                    # deferred queue): by then the exp stream has caught up,
                    # so the in-order PE queue never waits at pair boundaries
                    bb0, pt0 = pend.pop(0)
                    av_unit(a, ht, bb0, nchunks, oTs, pt0)

                    def tail_av(a_=a, ht_=ht, p_=tuple(pend), n_=nchunks,
                                o_=oTs):
                        for bb, pt in p_:
                            av_unit(a_, ht_, bb, n_, o_, pt)

                    def drain(a_=a, ht_=ht, o_=oTs):
                        for hh in range(2):
                            oT_sb = osb.tile([HD + 1, 512], BF, tag="oTsb",
                                             name="oTsb")
                            nc.vector.tensor_copy(oT_sb[:, :], o_[hh][:, :])
                            deferred.append(
                                (lambda hh_=hh, t_=oT_sb:
                                 finish_head(a_, ht_, hh_, t_)))

                        # store this head pair once its finishes have run
                        def store():
                            nc.sync.dma_start(
                                out[a_ * 512:(a_ + 1) * 512,
                                    ht_ * 128:(ht_ + 1) * 128].rearrange(
                                    "(c p) j -> p c j", p=128),
                                onat_by_a[a_][:, :].rearrange(
                                    "p (c j) -> p c j", c=4)[
                                    :, :, ht_ * 128:(ht_ + 1) * 128],
                            )
                        deferred.append(store)

                    deferred.insert(0, drain)
                    deferred.insert(0, tail_av)
            while deferred:
                deferred.pop(0)()

    nc.compile()
    return nc


def _get_nc():
    global _NC
    if _NC is None:
        _NC = _build()
    return _NC


def make_in_maps(hidden_states, Wqk, bqk, Wv, bv):
    from ml_dtypes import bfloat16

    x = np.asarray(hidden_states, dtype=np.float32)
    Wqk = np.asarray(Wqk, dtype=np.float32)
    bqk = np.asarray(bqk, dtype=np.float32)
    Wv = np.asarray(Wv, dtype=np.float32)
    bv = np.asarray(bv, dtype=np.float32)

    def pack(w):
        # [1024, C] -> SBUF image [128, 8*C] (k-chunk-major columns)
        c = w.shape[1]
        return np.ascontiguousarray(
            w.reshape(NK, 128, c).transpose(1, 0, 2).reshape(128, NK * c)
        ).astype(bfloat16)

    ident = np.eye(128, dtype=bfloat16)
    tri = np.triu(np.ones((128, 128), np.float32)).astype(bfloat16)
    # x quarters as SBUF images: xq[q][p, k*512+j] = x[b].T[k*128+p, q*512+j]
    xqs = []
    for b in range(B):
        xT = x[b].T.reshape(NK, 128, 4, 512)
        xqs.append([np.ascontiguousarray(
            xT[:, :, q, :].transpose(1, 0, 2).reshape(128, NK * 512)
        ).astype(bfloat16) for q in range(4)])
    in_maps = []
    for c in range(8):
        b, ho = c // 4, (c % 4) * NHL
        cols = slice(ho * HD, (ho + NHL) * HD)
        wv_aug = np.zeros((HID, VC), np.float32)
        wvl = np.zeros((1, VC), np.float32)
        for h in range(NHL):
            wv_aug[:, h * 65:h * 65 + HD] = Wv[:, (ho + h) * HD:(ho + h + 1) * HD]
            wvl[0, h * 65:h * 65 + HD] = bv[(ho + h) * HD:(ho + h + 1) * HD]
            wvl[0, h * 65 + HD] = 1.0
        bqk_c = np.stack([bqk[:HID][cols][:128], bqk[:HID][cols][128:],
                          bqk[HID:][cols][:128], bqk[HID:][cols][128:]],
                         axis=1)
        m = {
            "wq": pack(Wqk[:, cols]),
            "wk": pack(Wqk[:, HID:][:, cols]),
            "wv": pack(wv_aug),
            "wvl": wvl.astype(bfloat16),
            "bqk": np.ascontiguousarray(bqk_c.astype(np.float32)),
            "ident": ident,
            "tri": tri,
        }
        for q in range(4):
            m[f"xq{q}"] = xqs[b][q]
        in_maps.append(m)
    return in_maps


def kernel(hidden_states, Wqk, bqk, Wv, bv):
    import time

    from concourse.bass_utils import run_bass_kernel_spmd

    in_maps = make_in_maps(hidden_states, Wqk, bqk, Wv, bv)
    res = None
    for attempt in range(3):
        try:
            res = run_bass_kernel_spmd(_get_nc(), in_maps, list(range(8)))
            break
        except Exception:
            # transient NRT_EXEC_UNIT_UNRECOVERABLE errors have been observed
            # on this fabric; back off and retry
            if attempt == 2:
                raise
            time.sleep(2.0)
    outp = np.empty((B, S, NH * HD), np.float32)
    for c in range(8):
        b, ho = c // 4, (c % 4) * NHL
        outp[b, :, ho * HD:(ho + NHL) * HD] = res.results[c]["out"]
    return outp
